# revision 1
# baseline (speedup 1.0000x reference)
"""Trainium2 Bass kernel for nn_EpisodicMemoryModule.

Strategy
--------
Math restructure: inside each episode's scan, the gate chain
z -> g1 -> g depends only on (c_t, m, q) -- never on h. So G[t] for all
timesteps is precomputed with large batched matmuls, as is the
x-dependent half of the attention GRU (gi_att = c @ att_Wih.T, which is
even episode-invariant). The only sequential work left per scan step is
gh = h @ att_Whh.T plus elementwise gates.

Sharding: data-parallel over batch B=64 across 8 cores (8 rows/core),
no inter-core communication. All weights are pre-transposed on the host
into contraction-major layout (features on partitions) and pre-cast:
bf16 for everything except att_Whh.T which is fp8e4m3 scaled by 32
(the GRU recurrence is contractive; end-to-end rel err ~1e-3).

Within a core everything is feature-major ([d partitions, (t,b) free]),
which keeps the per-step gate elementwise work on fully-occupied
128-partition tiles.
"""

import sys

sys.path.insert(0, "/opt/trn_rl_repo")

import numpy as np
import ml_dtypes

import concourse.bass as bass
import concourse.mybir as mybir
from concourse.bass_utils import run_bass_kernel_spmd
from concourse.tile import TileContext
import bass_rust
from bass_rust import ScopedClock

T, B, D = 128, 64, 1024
NCORES = 8
BL = B // NCORES          # 8 batch rows per core
ROWS = T * BL             # 1024 rows per core
RB = 512                  # row-block for precompute matmuls
NRB = ROWS // RB          # 2
KC = D // 128             # 8 contraction chunks
S_WHH = 32.0              # fp8 scale for att_Whh

F32 = mybir.dt.float32
BF16 = mybir.dt.bfloat16
FP8 = mybir.dt.float8e4
AF = mybir.ActivationFunctionType
ALU = mybir.AluOpType

bf16_np = ml_dtypes.bfloat16
fp8_np = ml_dtypes.float8_e4m3fn


class _TC(TileContext):
    """TileContext whose final drain splits multi-sem waits (the walrus in
    this environment accepts only one sync wait per instruction)."""

    def _drain_and_barrier(self, tick_clock, wait_clock):
        drain_inst = self.nc.sync.drain()
        wait_clock.add_sem_waits(
            drain_inst.ins, ScopedClock({None: tick_clock.global_clock})
        )
        si = drain_inst.ins.sync_info
        if si is not None and si.on_wait and len(si.on_wait) > 1:
            waits = list(si.on_wait)
            drain_inst.ins.sync_info = bass_rust.SyncInfo(
                on_wait=[waits[0]], on_update=list(si.on_update or [])
            )
            for w in waits[1:]:
                d = self.nc.sync.drain()
                d.ins.sync_info = bass_rust.SyncInfo(on_wait=[w], on_update=[])
        self.nc.all_engine_barrier()
        assert self.sems is not None
        popped = self.nc._tile_sem_poison_stack.pop()
        assert popped is self._sem_poison
        self.nc.clear_and_free_semaphores(list(self.sems.allocated().values()))
        self.nc.all_engine_barrier()


def _split_multiwait(nc):
    """Split >1-wait instructions into single-wait NoOps + instruction."""
    nfix = 0
    for f in nc.m.functions:
        for bb in f.blocks:
            insts = list(bb.instructions)
            out = []
            changed = False
            for inst in insts:
                si = inst.sync_info
                if si and si.on_wait and len(si.on_wait) > 1:
                    waits = list(si.on_wait)
                    for i, w in enumerate(waits[:-1]):
                        nop = mybir.InstNoOp(
                            name=f"I-waitfix-{nfix}-{i}", ins=[], outs=[]
                        )
                        nop.engine = inst.engine
                        nop.sync_info = bass_rust.SyncInfo(on_wait=[w], on_update=[])
                        out.append(nop)
                    inst.sync_info = bass_rust.SyncInfo(
                        on_wait=[waits[-1]], on_update=list(si.on_update or [])
                    )
                    nfix += 1
                    changed = True
                out.append(inst)
            if changed:
                bb.instructions = out
    return nfix


def _build(t_steps=T, split_waits=True, phases=('gi', 'P', 'zg', 'scan', 'mem', 'small')):
    """Build the per-core Bass module (SPMD; every core runs the same
    program on its own batch shard)."""
    nc = bass.Bass()
    P = nc.declare_dram_parameter

    # Per-core activations (feature-major) and vectors.
    cT = P("cT", [D, ROWS], F32, isOutput=False)            # c.T shard
    qT = P("qT", [128, KC * BL], F32, isOutput=False)       # q.T folded (p,(k,b))
    # Weights, contraction-major (in-features on partitions).
    w1T = P("w1T", [9 * D, D], BF16, isOutput=False)        # W1.T
    w12T = P("w12T", [D, D], BF16, isOutput=False)          # (W1_1+W1_2).T
    wbT = P("wbT", [D, D], BF16, isOutput=False)            # Wb.T
    w2T = P("w2T", [D, D], BF16, isOutput=False)            # W2.T
    aWihT = P("aWihT", [D, 3 * D], BF16, isOutput=False)    # att_Wih.T
    aWhhT8 = P("aWhhT8", [D, 3 * D], FP8, isOutput=False)   # att_Whh.T * 32 fp8
    mWihT = P("mWihT", [D, 3 * D], BF16, isOutput=False)
    mWhhT = P("mWhhT", [D, 3 * D], BF16, isOutput=False)
    bZ = P("bZ", [128, KC], F32, isOutput=False)            # W1_b per (p, m)
    bG = P("bG", [128, KC], F32, isOutput=False)            # W2_b
    bA = P("bA", [128, 3 * KC], F32, isOutput=False)        # att bih(+bhh for r,z)
    bM = P("bM", [128, 3 * KC], F32, isOutput=False)        # mem bih(+bhh for r,z)
    out = P("out", [128, KC * BL], F32, isOutput=True)      # m2.T folded

    with _TC(nc) as tc:
        pool = tc.alloc_tile_pool(name="res", bufs=1)
        stream = tc.alloc_tile_pool(name="stream", bufs=6)
        scratch = tc.alloc_tile_pool(name="scratch", bufs=3)

        # ---- resident loads -------------------------------------------------
        cT_sb = pool.tile([128, KC * ROWS], F32, tag="cT")     # 32 KB/par
        for k in range(KC):
            nc.sync.dma_start(
                out=cT_sb[:, k * ROWS:(k + 1) * ROWS],
                in_=cT[k * 128:(k + 1) * 128, :],
            )
        qT_sb = pool.tile([128, KC * BL], F32, tag="qT")
        nc.sync.dma_start(out=qT_sb[:, :], in_=qT[:, :])
        whh_sb = pool.tile([128, KC * 3 * D], FP8, tag="whh")  # 24 KB/par
        for k in range(KC):
            nc.sync.dma_start(
                out=whh_sb[:, k * 3 * D:(k + 1) * 3 * D],
                in_=aWhhT8[k * 128:(k + 1) * 128, :],
            )
        bZ_sb = pool.tile([128, KC], F32, tag="bZ")
        nc.sync.dma_start(out=bZ_sb[:, :], in_=bZ[:, :])
        bG_sb = pool.tile([128, KC], F32, tag="bG")
        nc.sync.dma_start(out=bG_sb[:, :], in_=bG[:, :])
        bA_sb = pool.tile([128, 3 * KC], F32, tag="bA")
        nc.sync.dma_start(out=bA_sb[:, :], in_=bA[:, :])
        bM_sb = pool.tile([128, 3 * KC], F32, tag="bM")
        nc.sync.dma_start(out=bM_sb[:, :], in_=bM[:, :])

        qb_sb = pool.tile([128, KC * BL], BF16, tag="qb")
        nc.vector.tensor_copy(qb_sb[:, :], qT_sb[:, :])

        gi_sb = pool.tile([128, 3 * KC * ROWS], BF16, tag="gi")  # 48 KB/par
        p_sb = pool.tile([128, KC * ROWS], BF16, tag="P")        # 16 KB/par
        g_sb = pool.tile([128, KC * ROWS], BF16, tag="G")        # 16 KB/par

        def small_matmul(wT_dram, vec_sb, out_sb, bias=None, accum_from=None,
                         tagp="smallp"):
            """out.T[dout, BL] = W @ vec  (all feature-major [128, KC*BL]).
            One psum bank, k-outer single accumulation group, chunked weight
            DMA, fused bias/accum adds."""
            ps = tc.alloc_tile_pool(name="smallps", bufs=1, space="PSUM")
            pt = ps.tile([128, KC * BL], F32, tag=tagp, name=f"spt_{tagp}")
            for k in range(KC):
                wt = stream.tile([128, D], BF16, tag="w1w", name=f"sw_{tagp}{k}")
                nc.sync.dma_start(
                    out=wt[:, :], in_=wT_dram[k * 128:(k + 1) * 128, :])
                for m in range(KC):
                    nc.tensor.matmul(
                        pt[:, m * BL:(m + 1) * BL],
                        wt[:, m * 128:(m + 1) * 128],
                        vec_sb[:, k * BL:(k + 1) * BL],
                        start=(k == 0 and m == 0),
                        stop=(k == KC - 1 and m == KC - 1),
                    )
            src0 = pt[:, :]
            if bias is not None:
                nc.vector.tensor_tensor(
                    out_sb[:, :].rearrange("p (m b) -> p m b", b=BL),
                    src0.rearrange("p (m b) -> p m b", b=BL),
                    bias[:, :].unsqueeze(2).broadcast_to([128, KC, BL]),
                    ALU.add)
                src0 = out_sb[:, :]
            if accum_from is not None:
                nc.vector.tensor_tensor(
                    out_sb[:, :], src0, accum_from[:, :], ALU.add)
            elif bias is None:
                nc.vector.tensor_copy(out_sb[:, :], src0)
            ps.release()

        # ---- Wbq = q @ Wb.T ; qc1 = q @ (W1_1+W1_2).T + W1_b ---------------
        wbq_sb = pool.tile([128, KC * BL], F32, tag="wbq")
        qc1_sb = pool.tile([128, KC * BL], F32, tag="qc1")
        if 'small' in phases:
            small_matmul(wbT, qb_sb, wbq_sb)
            small_matmul(w12T, qb_sb, qc1_sb, bias=bZ_sb)
        else:
            nc.vector.memset(wbq_sb[:, :], 0.0)
            nc.vector.memset(qc1_sb[:, :], 0.0)

        def cast_c(k, rb):
            c8 = scratch.tile([128, RB], BF16, tag="c8", bufs=2,
                              name=f"c8_{k}_{rb}")
            nc.vector.tensor_copy(
                c8[:, :], cT_sb[:, k * ROWS + rb * RB: k * ROWS + (rb + 1) * RB])
            return c8

        def make_blocks(k, rb, vecf_sb, wbv_sb, tagsfx):
            """m-dependent blocks (c*v, |c-v|, c*Wbv) for chunk k, row-block
            rb; bf16 [128, RB] each."""
            cslice = cT_sb[:, k * ROWS + rb * RB: k * ROWS + (rb + 1) * RB]
            cview = cslice.rearrange("p (t b) -> p t b", b=BL)
            vv = vecf_sb[:, k * BL:(k + 1) * BL].unsqueeze(1).broadcast_to(
                [128, RB // BL, BL])
            wv = wbv_sb[:, k * BL:(k + 1) * BL].unsqueeze(1).broadcast_to(
                [128, RB // BL, BL])
            cm = scratch.tile([128, RB], BF16, tag="blk_cm", bufs=2,
                              name=f"cm{tagsfx}")
            nc.vector.tensor_tensor(
                cm[:, :].rearrange("p (t b) -> p t b", b=BL), cview, vv, ALU.mult)
            tmp = scratch.tile([128, RB], F32, tag="blk_tmp", bufs=2,
                               name=f"bt{tagsfx}")
            nc.vector.tensor_tensor(
                tmp[:, :].rearrange("p (t b) -> p t b", b=BL), cview, vv,
                ALU.subtract)
            am = scratch.tile([128, RB], BF16, tag="blk_am", bufs=2,
                              name=f"am{tagsfx}")
            nc.scalar.activation(am[:, :], tmp[:, :], AF.Abs)
            wm = scratch.tile([128, RB], BF16, tag="blk_wm", bufs=2,
                              name=f"wm{tagsfx}")
            nc.vector.tensor_tensor(
                wm[:, :].rearrange("p (t b) -> p t b", b=BL), cview, wv, ALU.mult)
            return cm, am, wm

        def pass_P(vecf_sb, wbv_sb, ps):
            """Shared Z part: P = c@W1_0.T + (c*q)@W1_3.T + |c-q|@W1_5.T
            + (c*Wbq)@W1_7.T  -> p_sb (bf16)."""
            for rb in range(NRB):
                zps = [ps.tile([128, RB], F32, tag="zp", name=f"pp{rb}_{m}")
                       for m in range(KC)]
                for k in range(KC):
                    c8 = cast_c(k, rb)
                    cm, am, wm = make_blocks(k, rb, vecf_sb, wbv_sb,
                                             f"P{rb}_{k}")
                    for ji, (j, blk) in enumerate(
                            [(0, c8), (3, cm), (5, am), (7, wm)]):
                        wt = stream.tile([128, D], BF16, tag="w1w",
                                         name=f"wP{rb}_{k}_{j}")
                        nc.sync.dma_start(
                            out=wt[:, :],
                            in_=w1T[j * D + k * 128: j * D + (k + 1) * 128, :])
                        for m in range(KC):
                            nc.tensor.matmul(
                                zps[m][:, :], wt[:, m * 128:(m + 1) * 128],
                                blk[:, :], start=(k == 0 and ji == 0),
                                stop=(k == KC - 1 and ji == 3))
                for m in range(KC):
                    nc.scalar.activation(
                        p_sb[:, m * ROWS + rb * RB: m * ROWS + (rb + 1) * RB],
                        zps[m][:, :], AF.Copy)

        def z_g_phase(vecf_sb, wbv_sb, qc_sb, tagsfx, ps=None):
            """Z-delta + P + qc -> tanh -> W2 -> sigmoid -> g_sb."""
            own = ps is None
            if own:
                ps = tc.alloc_tile_pool(name="zps", bufs=8, space="PSUM")
            for rb in range(NRB):
                g1s = [scratch.tile([128, RB], BF16, tag=f"g1_{m}", bufs=1,
                                    name=f"g1_{tagsfx}{rb}_{m}")
                       for m in range(KC)]
                zps = [ps.tile([128, RB], F32, tag="zp", name=f"zd{rb}_{m}")
                       for m in range(KC)]
                for k in range(KC):
                    cm, am, wm = make_blocks(k, rb, vecf_sb, wbv_sb,
                                             f"D{tagsfx}{rb}_{k}")
                    for ji, (j, blk) in enumerate([(4, cm), (6, am), (8, wm)]):
                        wt = stream.tile([128, D], BF16, tag="w1w",
                                         name=f"wD{tagsfx}{rb}_{k}_{j}")
                        nc.sync.dma_start(
                            out=wt[:, :],
                            in_=w1T[j * D + k * 128: j * D + (k + 1) * 128, :])
                        for m in range(KC):
                            nc.tensor.matmul(
                                zps[m][:, :], wt[:, m * 128:(m + 1) * 128],
                                blk[:, :], start=(k == 0 and ji == 0),
                                stop=(k == KC - 1 and ji == 2))
                for m in range(KC):
                    t1 = scratch.tile([128, RB], F32, tag="t1", bufs=2,
                                      name=f"t1_{tagsfx}{rb}_{m}")
                    nc.vector.scalar_tensor_tensor(
                        t1[:, :].rearrange("p (t b) -> p t b", b=BL),
                        zps[m][:, :].rearrange("p (t b) -> p t b", b=BL),
                        1.0,
                        qc_sb[:, m * BL:(m + 1) * BL].unsqueeze(1).broadcast_to(
                            [128, RB // BL, BL]),
                        ALU.mult, ALU.add)
                    t2 = scratch.tile([128, RB], F32, tag="t2", bufs=2,
                                      name=f"t2_{tagsfx}{rb}_{m}")
                    nc.vector.tensor_tensor(
                        t2[:, :], t1[:, :],
                        p_sb[:, m * ROWS + rb * RB: m * ROWS + (rb + 1) * RB],
                        ALU.add)
                    nc.scalar.activation(g1s[m][:, :], t2[:, :], AF.Tanh)
                # W2 phase (same pool/tag: slots rotate, no pool barrier)
                gps = [ps.tile([128, RB], F32, tag="zp", name=f"gp{rb}_{m}")
                       for m in range(KC)]
                for k in range(KC):
                    wt = stream.tile([128, D], BF16, tag="w1w",
                                     name=f"w2_{tagsfx}{rb}_{k}")
                    nc.sync.dma_start(
                        out=wt[:, :], in_=w2T[k * 128:(k + 1) * 128, :])
                    for m in range(KC):
                        nc.tensor.matmul(
                            gps[m][:, :], wt[:, m * 128:(m + 1) * 128],
                            g1s[k][:, :],
                            start=(k == 0), stop=(k == KC - 1))
                for m in range(KC):
                    nc.scalar.activation(
                        g_sb[:, m * ROWS + rb * RB: m * ROWS + (rb + 1) * RB],
                        gps[m][:, :], AF.Sigmoid, bias=bG_sb[:, m:m + 1])
            if own:
                ps.release()

        def gi_att_phase(ps):
            """gi_att = c @ att_Wih.T + bias, for all 24 out-chunks."""
            for mg in range(6):
                pts = {}
                for rb in range(NRB):
                    for mi in range(4):
                        pts[(rb, mi)] = ps.tile(
                            [128, RB], F32, tag="zp", name=f"gip{mg}_{rb}{mi}")
                for k in range(KC):
                    wt = stream.tile([128, 512], BF16, tag="wihw",
                                     name=f"wih{mg}_{k}")
                    nc.sync.dma_start(
                        out=wt[:, :],
                        in_=aWihT[k * 128:(k + 1) * 128,
                                  mg * 512:(mg + 1) * 512])
                    c8b = scratch.tile([128, ROWS], BF16, tag="c8f", bufs=2,
                                       name=f"c8f{mg}_{k}")
                    nc.vector.tensor_copy(
                        c8b[:, :], cT_sb[:, k * ROWS:(k + 1) * ROWS])
                    for rb in range(NRB):
                        for mi in range(4):
                            nc.tensor.matmul(
                                pts[(rb, mi)][:, :],
                                wt[:, mi * 128:(mi + 1) * 128],
                                c8b[:, rb * RB:(rb + 1) * RB],
                                start=(k == 0), stop=(k == KC - 1))
                for rb in range(NRB):
                    for mi in range(4):
                        m = mg * 4 + mi
                        nc.scalar.activation(
                            gi_sb[:, m * ROWS + rb * RB:
                                  m * ROWS + (rb + 1) * RB],
                            pts[(rb, mi)][:, :], AF.Identity,
                            bias=bA_sb[:, m:m + 1])

        def scan(nsteps, sfx):
            """Attention-GRU scan; returns final h (bf16, [128, KC*BL])."""
            ps = tc.alloc_tile_pool(name="scanps", bufs=2, space="PSUM")
            h8 = scratch.tile([128, KC * BL], BF16, tag="h8", name=f"h8{sfx}")
            nc.vector.memset(h8[:, :], 0.0)
            h = h8
            gi_v = gi_sb[:, :].rearrange(
                "p (m t b) -> p m t b", m=3 * KC, b=BL)
            g_v = g_sb[:, :].rearrange("p (m t b) -> p m t b", m=KC, b=BL)
            for t in range(nsteps):
                prz = ps.tile([128, 2 * KC * BL], F32, tag="ghprz",
                              name=f"ghprz{sfx}_{t}")
                pn = ps.tile([128, KC * BL], F32, tag="ghpn",
                             name=f"ghpn{sfx}_{t}")
                # r,z matmuls first so gate math overlaps the n matmuls
                for m in range(2 * KC):
                    for k in range(KC):
                        nc.tensor.matmul(
                            prz[:, m * BL:(m + 1) * BL],
                            whh_sb[:, k * 3 * D + m * 128:
                                   k * 3 * D + (m + 1) * 128],
                            h8[:, k * BL:(k + 1) * BL],
                            start=(k == 0), stop=(k == KC - 1))
                for mn in range(KC):
                    m = 2 * KC + mn
                    for k in range(KC):
                        nc.tensor.matmul(
                            pn[:, mn * BL:(mn + 1) * BL],
                            whh_sb[:, k * 3 * D + m * 128:
                                   k * 3 * D + (m + 1) * 128],
                            h8[:, k * BL:(k + 1) * BL],
                            start=(k == 0), stop=(k == KC - 1))
                # gates (feature-major): r first so the n-path starts
                # before the z half is through the DVE/ACT queues.
                nb_ = KC * BL
                rr = scratch.tile([128, nb_], F32, tag="rr",
                                  name=f"rr{sfx}_{t}")
                nc.vector.scalar_tensor_tensor(
                    rr[:, :].rearrange("p (m b) -> p m b", b=BL),
                    prz[:, 0:nb_].rearrange("p (m b) -> p m b", b=BL),
                    1.0 / S_WHH,
                    gi_v[:, 0:KC, t, :], ALU.mult, ALU.add)
                rs = scratch.tile([128, nb_], F32, tag="rs",
                                  name=f"rs{sfx}_{t}")
                nc.scalar.activation(rs[:, :], rr[:, :], AF.Sigmoid)
                zz = scratch.tile([128, nb_], F32, tag="zz",
                                  name=f"zz{sfx}_{t}")
                nc.vector.scalar_tensor_tensor(
                    zz[:, :].rearrange("p (m b) -> p m b", b=BL),
                    prz[:, nb_:2 * nb_].rearrange("p (m b) -> p m b", b=BL),
                    1.0 / S_WHH,
                    gi_v[:, KC:2 * KC, t, :], ALU.mult, ALU.add)
                zs = scratch.tile([128, nb_], F32, tag="zs",
                                  name=f"zs{sfx}_{t}")
                nc.scalar.activation(zs[:, :], zz[:, :], AF.Sigmoid)
                n1 = scratch.tile([128, nb_], F32, tag="n1",
                                  name=f"n1{sfx}_{t}")
                nc.vector.scalar_tensor_tensor(
                    n1[:, :], pn[:, :], 1.0 / S_WHH,
                    rs[:, :], ALU.mult, ALU.mult)
                n2 = scratch.tile([128, KC * BL], F32, tag="n2",
                                  name=f"n2{sfx}_{t}")
                nc.vector.tensor_tensor(
                    n2[:, :].rearrange("p (m b) -> p m b", b=BL),
                    n1[:, :].rearrange("p (m b) -> p m b", b=BL),
                    gi_v[:, 2 * KC:3 * KC, t, :], ALU.add)
                nt = scratch.tile([128, KC * BL], F32, tag="nt",
                                  name=f"nt{sfx}_{t}")
                nc.scalar.activation(nt[:, :], n2[:, :], AF.Tanh)
                # G' = G_t * (1 - z)   (parallel with tanh)
                oz = scratch.tile([128, KC * BL], F32, tag="oz",
                                  name=f"oz{sfx}_{t}")
                nc.vector.tensor_scalar(
                    oz[:, :], zs[:, :], -1.0, 1.0,
                    ALU.mult, ALU.add)
                gp = scratch.tile([128, KC * BL], F32, tag="gp2",
                                  name=f"gp{sfx}_{t}")
                nc.vector.tensor_tensor(
                    gp[:, :].rearrange("p (m b) -> p m b", b=BL),
                    oz[:, :].rearrange("p (m b) -> p m b", b=BL),
                    g_v[:, :, t, :], ALU.mult)
                # h_new = h + G'*(n - h), carried in bf16
                w1_ = scratch.tile([128, KC * BL], F32, tag="wd",
                                   name=f"wd{sfx}_{t}")
                nc.vector.tensor_tensor(w1_[:, :], nt[:, :], h[:, :],
                                        ALU.subtract)
                w2_ = scratch.tile([128, KC * BL], F32, tag="wu",
                                   name=f"wu{sfx}_{t}")
                nc.vector.tensor_tensor(w2_[:, :], w1_[:, :], gp[:, :], ALU.mult)
                hn = scratch.tile([128, KC * BL], BF16, tag="h8",
                                  name=f"h{sfx}_{t}")
                nc.vector.tensor_tensor(hn[:, :], w2_[:, :], h[:, :], ALU.add)
                h = hn
                h8 = hn
            ps.release()
            return h

        def mem_gru(e8_sb, m_sb, m8_sb, sfx):
            """m_new = GRUCell(e, m) with mem weights; feature-major."""
            ps = tc.alloc_tile_pool(name="memps", bufs=2, space="PSUM")
            nb = KC * BL
            gi_p = ps.tile([128, 3 * nb], F32, tag="memgh", name=f"mgi{sfx}")
            for k in range(KC):
                for g3 in range(3):
                    wt = stream.tile([128, D], BF16, tag="w1w",
                                     name=f"mw{sfx}_{k}_{g3}")
                    nc.sync.dma_start(
                        out=wt[:, :],
                        in_=mWihT[k * 128:(k + 1) * 128,
                                  g3 * D:(g3 + 1) * D])
                    for mm in range(KC):
                        m = g3 * KC + mm
                        nc.tensor.matmul(
                            gi_p[:, m * BL:(m + 1) * BL],
                            wt[:, mm * 128:(mm + 1) * 128],
                            e8_sb[:, k * BL:(k + 1) * BL],
                            start=(k == 0 and g3 == 0 and mm == 0),
                            stop=(k == KC - 1 and g3 == 2 and mm == KC - 1))
            gi_f = scratch.tile([128, 3 * nb], F32, tag="memgif",
                                name=f"mgif{sfx}")
            nc.vector.tensor_tensor(
                gi_f[:, :].rearrange("p (m b) -> p m b", b=BL),
                gi_p[:, :].rearrange("p (m b) -> p m b", b=BL),
                bM_sb[:, :].unsqueeze(2).broadcast_to([128, 3 * KC, BL]),
                ALU.add)
            gh_p = ps.tile([128, 3 * nb], F32, tag="memgh", name=f"mgh{sfx}")
            for k in range(KC):
                for g3 in range(3):
                    wt = stream.tile([128, D], BF16, tag="w1w",
                                     name=f"mwh{sfx}_{k}_{g3}")
                    nc.sync.dma_start(
                        out=wt[:, :],
                        in_=mWhhT[k * 128:(k + 1) * 128,
                                  g3 * D:(g3 + 1) * D])
                    for mm in range(KC):
                        m = g3 * KC + mm
                        nc.tensor.matmul(
                            gh_p[:, m * BL:(m + 1) * BL],
                            wt[:, mm * 128:(mm + 1) * 128],
                            m8_sb[:, k * BL:(k + 1) * BL],
                            start=(k == 0 and g3 == 0 and mm == 0),
                            stop=(k == KC - 1 and g3 == 2 and mm == KC - 1))
            rz = scratch.tile([128, 2 * nb], F32, tag="mrz", name=f"mrz{sfx}")
            nc.vector.tensor_tensor(
                rz[:, :], gi_f[:, 0:2 * nb], gh_p[:, 0:2 * nb], ALU.add)
            rzs = scratch.tile([128, 2 * nb], F32, tag="mrzs", name=f"mrzs{sfx}")
            nc.scalar.activation(rzs[:, :], rz[:, :], AF.Sigmoid)
            n1 = scratch.tile([128, nb], F32, tag="mn1", name=f"mn1{sfx}")
            nc.vector.tensor_tensor(
                n1[:, :], rzs[:, 0:nb], gh_p[:, 2 * nb:3 * nb], ALU.mult)
            n2 = scratch.tile([128, nb], F32, tag="mn2", name=f"mn2{sfx}")
            nc.vector.tensor_tensor(
                n2[:, :], n1[:, :], gi_f[:, 2 * nb:3 * nb], ALU.add)
            nt = scratch.tile([128, nb], F32, tag="mnt", name=f"mnt{sfx}")
            nc.scalar.activation(nt[:, :], n2[:, :], AF.Tanh)
            d1 = scratch.tile([128, nb], F32, tag="md1", name=f"md1{sfx}")
            nc.vector.tensor_tensor(d1[:, :], m_sb[:, :], nt[:, :],
                                    ALU.subtract)
            d2 = scratch.tile([128, nb], F32, tag="md2", name=f"md2{sfx}")
            nc.vector.tensor_tensor(d2[:, :], d1[:, :], rzs[:, nb:2 * nb],
                                    ALU.mult)
            mn = scratch.tile([128, nb], F32, tag="mnew", bufs=2,
                              name=f"mn{sfx}")
            nc.vector.tensor_tensor(mn[:, :], d2[:, :], nt[:, :], ALU.add)
            mn8 = scratch.tile([128, nb], BF16, tag="mnew8", bufs=2,
                               name=f"mn8{sfx}")
            nc.vector.tensor_copy(mn8[:, :], mn[:, :])
            ps.release()
            return mn, mn8

        # ================= episode 1 (m = q) =================
        ep1ps = tc.alloc_tile_pool(name="ep1ps", bufs=8, space="PSUM")
        if 'gi' in phases:
            gi_att_phase(ep1ps)
        if 'P' in phases:
            pass_P(qT_sb, wbq_sb, ep1ps)
        if 'zg' in phases:
            z_g_phase(qT_sb, wbq_sb, qc1_sb, "a", ps=ep1ps)
        ep1ps.release()
        if 'scan' in phases:
            h1 = scan(t_steps, "a")
        else:
            h1 = scratch.tile([128, KC * BL], BF16, tag="h8", name="hstub_a")
            nc.vector.memset(h1[:, :], 0.0)
        m1, m1_8 = mem_gru(h1, qT_sb, qb_sb, "a")

        # ================= episode 2 (m = m1) =================
        wbm_sb = pool.tile([128, KC * BL], F32, tag="wbm")
        if 'small' in phases:
            small_matmul(wbT, m1_8, wbm_sb)
        else:
            nc.vector.memset(wbm_sb[:, :], 0.0)

        class _W1Slice:
            """View of w1T rows [off, off+D) as a [D, D] dram tensor."""
            def __init__(self, off):
                self.off = off
            def __getitem__(self, idx):
                ksl, msl = idx
                return w1T[self.off + ksl.start: self.off + ksl.stop, msl]

        qc2a = pool.tile([128, KC * BL], F32, tag="qc2a")
        qc2 = pool.tile([128, KC * BL], F32, tag="qc2")
        if 'small' in phases:
            small_matmul(_W1Slice(1 * D), m1_8, qc2a)
            small_matmul(_W1Slice(2 * D), qb_sb, qc2, bias=bZ_sb,
                         accum_from=qc2a)
        else:
            nc.vector.memset(qc2[:, :], 0.0)

        if 'zg' in phases:
            z_g_phase(m1, wbm_sb, qc2, "b")
        if 'scan' in phases:
            h2 = scan(t_steps, "b")
        else:
            h2 = scratch.tile([128, KC * BL], BF16, tag="h8", name="hstub_b")
            nc.vector.memset(h2[:, :], 0.0)
        m2, _ = mem_gru(h2, m1, m1_8, "b")

        nc.sync.dma_start(out=out[:, :], in_=m2[:, :])

        for p in (scratch, stream, pool):
            p.release()

    if split_waits:
        _split_multiwait(nc)
    return nc


_cache = {}


def _get_nc(t_steps=T):
    if t_steps not in _cache:
        _cache[t_steps] = _build(t_steps)
    return _cache[t_steps]


def _prep_inputs(c, q, Wb_w, W1_w, W1_b, W2_w, W2_b,
                 mem_Wih, mem_Whh, mem_bih, mem_bhh,
                 att_Wih, att_Whh, att_bih, att_bhh):
    """Host-side: transpose/cast/shard everything into per-core in_maps."""
    f32 = np.float32
    c = np.asarray(c, f32); q = np.asarray(q, f32)
    W1j = [np.asarray(W1_w[:, j * D:(j + 1) * D], f32) for j in range(9)]

    def fold_bias(v):  # [D] -> [128, KC] (p, m)
        return np.ascontiguousarray(
            np.asarray(v, f32).reshape(KC, 128).T)

    def fold_bias3(bih, bhh):  # [3D] -> [128, 3KC]; bhh only for r,z
        v = np.asarray(bih, f32).copy()
        bhh = np.asarray(bhh, f32)
        v[:2 * D] += bhh[:2 * D]
        return np.ascontiguousarray(v.reshape(3 * KC, 128).T)

    shared = {
        "w1T": np.ascontiguousarray(np.asarray(W1_w, f32).T).astype(bf16_np),
        "w12T": np.ascontiguousarray((W1j[1] + W1j[2]).T).astype(bf16_np),
        "wbT": np.ascontiguousarray(np.asarray(Wb_w, f32).T).astype(bf16_np),
        "w2T": np.ascontiguousarray(np.asarray(W2_w, f32).T).astype(bf16_np),
        "aWihT": np.ascontiguousarray(np.asarray(att_Wih, f32).T).astype(bf16_np),
        "aWhhT8": (np.ascontiguousarray(np.asarray(att_Whh, f32).T)
                   * S_WHH).astype(fp8_np),
        "mWihT": np.ascontiguousarray(np.asarray(mem_Wih, f32).T).astype(bf16_np),
        "mWhhT": np.ascontiguousarray(np.asarray(mem_Whh, f32).T).astype(bf16_np),
        "bZ": fold_bias(W1_b),
        "bG": fold_bias(W2_b),
        "bA": fold_bias3(att_bih, att_bhh),
        "bM": fold_bias3(mem_bih, mem_bhh),
    }
    assert not np.any(np.asarray(att_bhh, f32)[2 * D:]), \
        "nonzero att_bhh n-gate bias not supported by this kernel build"
    assert not np.any(np.asarray(mem_bhh, f32)[2 * D:]), \
        "nonzero mem_bhh n-gate bias not supported by this kernel build"

    in_maps = []
    for ci in range(NCORES):
        s = ci * BL
        csh = c[:, s:s + BL, :].reshape(ROWS, D)
        qsh = q[s:s + BL, :]
        im = dict(shared)
        im["cT"] = np.ascontiguousarray(csh.T)
        im["qT"] = np.ascontiguousarray(
            qsh.reshape(BL, KC, 128).transpose(2, 1, 0).reshape(128, KC * BL))
        in_maps.append(im)
    return in_maps


def _unshard(results):
    m = np.empty((B, D), np.float32)
    for ci in range(NCORES):
        o = results[ci]["out"]  # [128, KC*BL]: [p, (k, b)]
        m[ci * BL:(ci + 1) * BL] = (
            o.reshape(128, KC, BL).transpose(2, 1, 0).reshape(BL, D))
    return m


def run_device(in_maps, trace=False):
    nc = _get_nc()
    res = run_bass_kernel_spmd(nc, in_maps, list(range(NCORES)), trace=trace)
    return res


def kernel(**inputs) -> np.ndarray:
    in_maps = _prep_inputs(**inputs)
    res = run_device(in_maps)
    return _unshard(res.results)


if __name__ == "__main__":
    np.random.seed(0)
    pass



# revision 7
# speedup vs baseline: 1.3778x; 1.3778x over previous
"""Trainium2 Bass kernel for nn_EpisodicMemoryModule.

Strategy (v2)
-------------
Math restructure: inside each episode's scan, the gate chain
z -> g1 -> g depends only on (c_t, m, q) -- never on h -- so G[t] is
precomputed with batched matmuls, as is gi_att = c @ att_Wih.T (episode
invariant).  The only sequential work per scan step is
gh = h @ att_Whh.T plus a short elementwise chain.

v2 changes vs v1:
 * All large matmuls run fp8e4m3 with DoubleRow perf mode (256-deep
   contraction per instruction, 0.5 cycles/row).  Weights are scaled by
   32 on the host; descales fold into ACT scale slots.
 * The scan step's per-step bias+gi term is seeded directly into PSUM
   with a single identity matmul, and the z-gate weights/gi are negated
   host-side so ONE sigmoid over the seeded bank yields both r and
   (1-z).  The per-step chain is:
     PE (96 DoubleRow matmuls) -> ACT sigmoid -> DVE n1,n2 -> ACT tanh
     -> DVE a1 -> DVE h'(fp8), with h'(bf16), gp, hgp, hmg off-path.
 * Episode-specific z-delta passes (and the shared P pass) are emitted
   as generators interleaved between scan steps, filling idle PE/DVE/ACT
   slots (the scan is latency-bound, ~35% engine busy).

Sharding: data-parallel over batch B=64 across 8 cores; no inter-core
communication.
"""

import sys

sys.path.insert(0, "/opt/trn_rl_repo")

import numpy as np
import ml_dtypes

import concourse.bass as bass
import concourse.mybir as mybir
from concourse.bass_utils import run_bass_kernel_spmd
from concourse.tile import TileContext
import bass_rust
from bass_rust import ScopedClock

T, B, D = 128, 64, 1024
NCORES = 8
BL = B // NCORES          # 8 batch rows per core
ROWS = T * BL             # 1024 rows per core
KC = D // 128             # 8 contraction chunks of 128
KC2 = D // 256            # 4 contraction chunks of 256 (DoubleRow)
M3 = 3 * D // 128         # 24 output chunks for 3D
RQ = 256                  # row-block for pipelined precompute matmuls
NRQ = ROWS // RQ          # 4
TQ = RQ // BL             # 32 timesteps per row-block
S = 32.0                  # fp8 weight scale

F32 = mybir.dt.float32
BF16 = mybir.dt.bfloat16
FP8 = mybir.dt.float8e4
AF = mybir.ActivationFunctionType
ALU = mybir.AluOpType
DR = mybir.MatmulPerfMode.DoubleRow

bf16_np = ml_dtypes.bfloat16
fp8_np = ml_dtypes.float8_e4m3fn

JP = [0, 3, 5, 7]         # episode-invariant W1 blocks: c, c*q, |c-q|, c*Wbq
JD = [4, 6, 8]            # episode-specific W1 blocks: c*m, |c-m|, c*Wbm


class _TC(TileContext):
    """TileContext whose final drain splits multi-sem waits (the walrus in
    this environment accepts only one sync wait per instruction)."""

    def _drain_and_barrier(self, tick_clock, wait_clock):
        drain_inst = self.nc.sync.drain()
        wait_clock.add_sem_waits(
            drain_inst.ins, ScopedClock({None: tick_clock.global_clock})
        )
        si = drain_inst.ins.sync_info
        if si is not None and si.on_wait and len(si.on_wait) > 1:
            waits = list(si.on_wait)
            drain_inst.ins.sync_info = bass_rust.SyncInfo(
                on_wait=[waits[0]], on_update=list(si.on_update or [])
            )
            for w in waits[1:]:
                d = self.nc.sync.drain()
                d.ins.sync_info = bass_rust.SyncInfo(on_wait=[w], on_update=[])
        self.nc.all_engine_barrier()
        assert self.sems is not None
        popped = self.nc._tile_sem_poison_stack.pop()
        assert popped is self._sem_poison
        self.nc.clear_and_free_semaphores(list(self.sems.allocated().values()))
        self.nc.all_engine_barrier()


def _split_multiwait(nc):
    """Split >1-wait instructions into single-wait NoOps + instruction."""
    nfix = 0
    for f in nc.m.functions:
        for bb in f.blocks:
            insts = list(bb.instructions)
            out = []
            changed = False
            for inst in insts:
                si = inst.sync_info
                if si and si.on_wait and len(si.on_wait) > 1:
                    waits = list(si.on_wait)
                    for i, w in enumerate(waits[:-1]):
                        nop = mybir.InstNoOp(
                            name=f"I-waitfix-{nfix}-{i}", ins=[], outs=[]
                        )
                        nop.engine = inst.engine
                        nop.sync_info = bass_rust.SyncInfo(on_wait=[w], on_update=[])
                        out.append(nop)
                    inst.sync_info = bass_rust.SyncInfo(
                        on_wait=[waits[-1]], on_update=list(si.on_update or [])
                    )
                    nfix += 1
                    changed = True
                out.append(inst)
            if changed:
                bb.instructions = out
    return nfix


def _build(t_steps=T, split_waits=True, pipeline=True):
    """Build the per-core Bass module (SPMD; every core runs the same
    program on its own batch shard)."""
    nc = bass.Bass()
    P = nc.declare_dram_parameter

    # Per-core activations (feature-major) and vectors.
    cT = P("cT", [128, KC * ROWS], BF16, isOutput=False)     # [p,(k rows)]
    qT = P("qT", [128, KC * BL], F32, isOutput=False)        # [p,(k b)]
    # fp8 weights, x32, DoubleRow layouts [p, (.. two ..)].
    w1P = P("w1P", [128, KC2 * len(JP) * 2 * D], FP8, isOutput=False)
    w1D = P("w1D", [128, KC2 * len(JD) * 2 * D], FP8, isOutput=False)
    wih2 = P("wih2", [128, KC2 * 2 * 3 * D], FP8, isOutput=False)
    whh2 = P("whh2", [128, KC2 * 2 * 3 * D], FP8, isOutput=False)  # z negated
    w22 = P("w22", [128, KC2 * 2 * D], FP8, isOutput=False)
    # bf16 weights for the small (BL-row) matmuls.
    wbT = P("wbT", [D, D], BF16, isOutput=False)             # Wb.T
    w12sT = P("w12sT", [D, D], BF16, isOutput=False)         # (W1_1+W1_2).T
    w11T = P("w11T", [D, D], BF16, isOutput=False)           # W1_1.T
    w12T = P("w12T", [D, D], BF16, isOutput=False)           # W1_2.T
    mWihT = P("mWihT", [D, 3 * D], BF16, isOutput=False)
    mWhhT = P("mWhhT", [D, 3 * D], BF16, isOutput=False)
    ident = P("ident", [128, 128], BF16, isOutput=False)
    bZ = P("bZ", [128, KC], F32, isOutput=False)             # W1_b per (p, m)
    bG = P("bG", [128, KC], F32, isOutput=False)             # W2_b
    bA32 = P("bA32", [128, 2 * KC], F32, isOutput=False)     # +-32*(bih+bhh) r,z
    bAn = P("bAn", [128, KC], F32, isOutput=False)           # bih n
    bM = P("bM", [128, 3 * KC], F32, isOutput=False)         # mem bih(+bhh r,z)
    out = P("out", [128, KC * BL], F32, isOutput=True)       # m2.T folded

    with _TC(nc) as tc:
        pool = tc.alloc_tile_pool(name="res", bufs=1)
        stream = tc.alloc_tile_pool(name="stream", bufs=2)
        scratch = tc.alloc_tile_pool(name="scratch", bufs=3)

        # ---- resident loads -------------------------------------------------
        cT_sb = pool.tile([128, KC * ROWS], BF16, tag="cT")    # 16 KB/par
        for k in range(KC):
            nc.sync.dma_start(
                out=cT_sb[:, k * ROWS:(k + 1) * ROWS],
                in_=cT[:, k * ROWS:(k + 1) * ROWS],
            )
        qT_sb = pool.tile([128, KC * BL], F32, tag="qT")
        nc.sync.dma_start(out=qT_sb[:, :], in_=qT[:, :])
        whh_sb = pool.tile([128, KC2 * 2 * 3 * D], FP8, tag="whh")  # 24 KB/par
        for k2 in range(KC2):
            nc.sync.dma_start(
                out=whh_sb[:, k2 * 6 * D:(k2 + 1) * 6 * D],
                in_=whh2[:, k2 * 6 * D:(k2 + 1) * 6 * D],
            )
        id_sb = pool.tile([128, 128], BF16, tag="id")
        nc.sync.dma_start(out=id_sb[:, :], in_=ident[:, :])
        bZ_sb = pool.tile([128, KC], F32, tag="bZ")
        nc.sync.dma_start(out=bZ_sb[:, :], in_=bZ[:, :])
        bG_sb = pool.tile([128, KC], F32, tag="bG")
        nc.sync.dma_start(out=bG_sb[:, :], in_=bG[:, :])
        bA_sb = pool.tile([128, 2 * KC], F32, tag="bA")
        nc.sync.dma_start(out=bA_sb[:, :], in_=bA32[:, :])
        bAn_sb = pool.tile([128, KC], F32, tag="bAn")
        nc.sync.dma_start(out=bAn_sb[:, :], in_=bAn[:, :])
        bM_sb = pool.tile([128, 3 * KC], F32, tag="bM")
        nc.sync.dma_start(out=bM_sb[:, :], in_=bM[:, :])

        qb_sb = pool.tile([128, KC * BL], BF16, tag="qb")
        nc.vector.tensor_copy(qb_sb[:, :], qT_sb[:, :])

        # fp8 copy of c (matmul rhs for gi / j0 blocks)
        c8_sb = pool.tile([128, KC * ROWS], FP8, tag="c8")     # 8 KB/par
        for k in range(KC):
            nc.vector.tensor_copy(
                c8_sb[:, k * ROWS:(k + 1) * ROWS],
                cT_sb[:, k * ROWS:(k + 1) * ROWS])

        # precomputed per-t data, t-major layouts
        gi_rz = pool.tile([128, T * 2 * KC * BL], BF16, tag="girz")  # 32 KB
        gi_n = pool.tile([128, T * KC * BL], BF16, tag="gin")        # 16 KB
        g_sb = pool.tile([128, T * KC * BL], BF16, tag="G")          # 16 KB
        p_sb = pool.tile([128, KC * ROWS], BF16, tag="P")            # 16 KB (32*P)

        def small_matmul(wT_dram, vec_sb, out_sb, bias=None, accum_from=None,
                         tagp="smallp"):
            """out.T[dout, BL] = W @ vec  (feature-major [128, KC*BL]).
            bf16, one psum accumulation group, chunked weight DMA."""
            ps = tc.alloc_tile_pool(name="smallps", bufs=1, space="PSUM")
            pt = ps.tile([128, KC * BL], F32, tag=tagp, name=f"spt_{tagp}")
            for k in range(KC):
                wt = stream.tile([128, D], BF16, tag="smw", name=f"sw_{tagp}{k}")
                nc.sync.dma_start(
                    out=wt[:, :], in_=wT_dram[k * 128:(k + 1) * 128, :])
                for m in range(KC):
                    nc.tensor.matmul(
                        pt[:, m * BL:(m + 1) * BL],
                        wt[:, m * 128:(m + 1) * 128],
                        vec_sb[:, k * BL:(k + 1) * BL],
                        start=(k == 0 and m == 0),
                        stop=(k == KC - 1 and m == KC - 1),
                    )
            src0 = pt[:, :]
            if bias is not None:
                nc.vector.tensor_tensor(
                    out_sb[:, :].rearrange("p (m b) -> p m b", b=BL),
                    src0.rearrange("p (m b) -> p m b", b=BL),
                    bias[:, :].unsqueeze(2).broadcast_to([128, KC, BL]),
                    ALU.add)
                src0 = out_sb[:, :]
            if accum_from is not None:
                nc.vector.tensor_tensor(
                    out_sb[:, :], src0, accum_from[:, :], ALU.add)
            elif bias is None:
                nc.vector.tensor_copy(out_sb[:, :], src0)
            ps.release()

        # ---- Wbq = q @ Wb.T ; qc1 = q @ (W1_1+W1_2).T + W1_b ---------------
        wbq_sb = pool.tile([128, KC * BL], F32, tag="wbq")
        qc1_sb = pool.tile([128, KC * BL], F32, tag="qc1")
        small_matmul(wbT, qb_sb, wbq_sb, tagp="swbq")
        small_matmul(w12sT, qb_sb, qc1_sb, bias=bZ_sb, tagp="sqc1")

        # ---- gi phase (lead-in, not pipelined) ------------------------------
        # gi_rz[p, (t, m2, b)] = +-32*(c@Wih.T + bA)[r|z], z negated.
        # gi_n[p, (t, k, b)] = (c@Wih.T + bih)[n].
        def gi_phase():
            ps = tc.alloc_tile_pool(name="gips", bufs=8, space="PSUM")
            c8v = c8_sb[:, :].rearrange("p (k r) -> p k r", k=KC)
            for mg in range(6):            # 4 out-chunks per group
                pts = {}
                for rb in range(2):
                    for mi in range(4):
                        pts[(rb, mi)] = ps.tile(
                            [128, 512], F32, tag="zp", name=f"gip{mg}_{rb}{mi}")
                for k2 in range(KC2):
                    wt = stream.tile([128, 2 * 512], FP8, tag="wih",
                                     name=f"wih{mg}_{k2}")
                    nc.sync.dma_start(
                        out=wt[:, :],
                        in_=wih2[:, (k2 * 6 + mg) * 1024:
                                 (k2 * 6 + mg + 1) * 1024])
                    wv = wt[:, :].rearrange("p (two m) -> p two m", two=2)
                    for rb in range(2):
                        for mi in range(4):
                            nc.tensor.matmul(
                                pts[(rb, mi)][:, :],
                                wv[:, :, mi * 128:(mi + 1) * 128],
                                c8v[:, 2 * k2:2 * k2 + 2,
                                    rb * 512:(rb + 1) * 512],
                                start=(k2 == 0), stop=(k2 == KC2 - 1),
                                perf_mode=DR)
                for rb in range(2):
                    for mi in range(4):
                        m = mg * 4 + mi
                        pv = pts[(rb, mi)][:, :].rearrange(
                            "p (t b) -> p t b", b=BL)
                        if m < 2 * KC:   # r,z: out = +-(psum + 32*bA)
                            sgn = 1.0 if m < KC else -1.0
                            ov = gi_rz[:, :].rearrange(
                                "p (t m b) -> p t m b", m=2 * KC, b=BL)
                            nc.scalar.activation(
                                ov[:, rb * 64:(rb + 1) * 64, m, :], pv,
                                AF.Identity, scale=sgn,
                                bias=bA_sb[:, m:m + 1])
                        else:            # n: out = psum/32 + bih_n
                            mn = m - 2 * KC
                            ov = gi_n[:, :].rearrange(
                                "p (t k b) -> p t k b", k=KC, b=BL)
                            nc.scalar.activation(
                                ov[:, rb * 64:(rb + 1) * 64, mn, :], pv,
                                AF.Identity, scale=1.0 / S,
                                bias=bAn_sb[:, mn:mn + 1])
            ps.release()

        gi_phase()

        # ---- pipelined P / z-delta passes ----------------------------------
        # Pair accumulators: [128, 512] f32 = one full PSUM bank per tile,
        # holding two adjacent 256-col m-chunks.  Every pair bank is seeded
        # by a single full-bank matmul (start=True) so the 2KB zero-region
        # semantics never poison a half that is mid-accumulation.
        pps = tc.alloc_tile_pool(name="pps", bufs=4, space="PSUM")
        zero_sb = pool.tile([128, 512], BF16, tag="zero")
        nc.vector.memset(zero_sb[:, :], 0.0)
        cT4 = cT_sb[:, :].rearrange("p (k t b) -> p k t b", k=KC, b=BL)
        c8v3 = c8_sb[:, :].rearrange("p (k r) -> p k r", k=KC)

        def make_block2(k2, rq, vecf_sb, wbv_sb, tagsfx):
            """fp8 feature block pairs for 256-chunk k2, row-block rq.
            Each is [128, 2*RQ] laid out (two, t, b)."""
            cview = cT4[:, 2 * k2:2 * k2 + 2, rq * TQ:(rq + 1) * TQ, :]
            vview = vecf_sb[:, :].rearrange("p (k b) -> p k b", b=BL)[
                :, 2 * k2:2 * k2 + 2, :].unsqueeze(2).broadcast_to(
                [128, 2, TQ, BL])
            wview = wbv_sb[:, :].rearrange("p (k b) -> p k b", b=BL)[
                :, 2 * k2:2 * k2 + 2, :].unsqueeze(2).broadcast_to(
                [128, 2, TQ, BL])
            cm = scratch.tile([128, 2 * RQ], FP8, tag="blk_cm", bufs=2,
                              name=f"cm{tagsfx}")
            nc.vector.tensor_tensor(
                cm[:, :].rearrange("p (two t b) -> p two t b", two=2, b=BL),
                cview, vview, ALU.mult)
            tmp = scratch.tile([128, 2 * RQ], BF16, tag="blk_tmp", bufs=2,
                               name=f"bt{tagsfx}")
            nc.vector.tensor_tensor(
                tmp[:, :].rearrange("p (two t b) -> p two t b", two=2, b=BL),
                cview, vview, ALU.subtract)
            am = scratch.tile([128, 2 * RQ], FP8, tag="blk_am", bufs=2,
                              name=f"am{tagsfx}")
            nc.scalar.activation(am[:, :], tmp[:, :], AF.Abs)
            wm = scratch.tile([128, 2 * RQ], FP8, tag="blk_wm", bufs=2,
                              name=f"wm{tagsfx}")
            nc.vector.tensor_tensor(
                wm[:, :].rearrange("p (two t b) -> p two t b", two=2, b=BL),
                cview, wview, ALU.mult)
            return cm, am, wm

        def _pair_rhs(blk):
            return blk[:, :].rearrange("p (two f) -> p two f", two=2)

        def P_phase(rq):
            """P[rq] = 32*(c@W1_0 + (c*q)@W1_3 + |c-q|@W1_5 + (c*Wbq)@W1_7)
            -> p_sb slice (bf16)."""
            zps2 = [pps.tile([128, 512], F32, tag="pzp", name=f"pp{rq}_{mg}")
                    for mg in range(KC // 2)]
            for mg in range(KC // 2):
                nc.tensor.matmul(zps2[mg][:, :], id_sb[:, :],
                                 zero_sb[:, :], start=True, stop=False,
                                 skip_group_check=True)
            yield
            for k2 in range(KC2):
                cm, am, wm = make_block2(k2, rq, qT_sb, wbq_sb, f"P{rq}_{k2}")
                yield
                wt = stream.tile([128, len(JP) * 2 * D], FP8, tag="w1p",
                                 name=f"wP{rq}_{k2}")
                nc.sync.dma_start(
                    out=wt[:, :],
                    in_=w1P[:, k2 * len(JP) * 2 * D:
                            (k2 + 1) * len(JP) * 2 * D])
                yield
                wv = wt[:, :].rearrange("p (j two m) -> p j two m",
                                        j=len(JP), two=2)
                rhss = [c8v3[:, 2 * k2:2 * k2 + 2, rq * RQ:(rq + 1) * RQ],
                        _pair_rhs(cm), _pair_rhs(am), _pair_rhs(wm)]
                for ji in range(len(JP)):
                    for m in range(KC):
                        nc.tensor.matmul(
                            zps2[m // 2][:, (m % 2) * RQ:(m % 2 + 1) * RQ],
                            wv[:, ji, :, m * 128:(m + 1) * 128],
                            rhss[ji],
                            start=False,
                            stop=(k2 == KC2 - 1 and ji == len(JP) - 1),
                            perf_mode=DR, skip_group_check=True)
                    yield
            for m in range(KC):
                nc.scalar.activation(
                    p_sb[:, m * ROWS + rq * RQ: m * ROWS + (rq + 1) * RQ],
                    zps2[m // 2][:, (m % 2) * RQ:(m % 2 + 1) * RQ], AF.Copy)
                yield

        def zg_phase(ep, rq, vecf_sb, wbv_sb, qc_sb):
            """G[rq] for episode ep -> g_sb (t-major slices)."""
            sfx = f"{ep}_{rq}"
            zps2 = [pps.tile([128, 512], F32, tag="pzp", name=f"zd{sfx}_{mg}")
                    for mg in range(KC // 2)]
            p3 = p_sb[:, :].rearrange("p (m r) -> p m r", m=KC)
            for mg in range(KC // 2):
                # seed pair bank with 32*P (one full-bank identity matmul)
                nc.tensor.matmul(
                    zps2[mg][:, :], id_sb[:, :],
                    p3[:, 2 * mg:2 * mg + 2, rq * RQ:(rq + 1) * RQ],
                    start=True, stop=False, skip_group_check=True)
            yield
            for k2 in range(KC2):
                cm, am, wm = make_block2(k2, rq, vecf_sb, wbv_sb,
                                         f"D{sfx}_{k2}")
                yield
                wt = stream.tile([128, len(JD) * 2 * D], FP8, tag="w1d",
                                 name=f"wD{sfx}_{k2}")
                nc.sync.dma_start(
                    out=wt[:, :],
                    in_=w1D[:, k2 * len(JD) * 2 * D:
                            (k2 + 1) * len(JD) * 2 * D])
                yield
                wv = wt[:, :].rearrange("p (j two m) -> p j two m",
                                        j=len(JD), two=2)
                rhss = [_pair_rhs(cm), _pair_rhs(am), _pair_rhs(wm)]
                for ji in range(len(JD)):
                    for m in range(KC):
                        nc.tensor.matmul(
                            zps2[m // 2][:, (m % 2) * RQ:(m % 2 + 1) * RQ],
                            wv[:, ji, :, m * 128:(m + 1) * 128],
                            rhss[ji],
                            start=False,
                            stop=(k2 == KC2 - 1 and ji == len(JD) - 1),
                            perf_mode=DR, skip_group_check=True)
                    yield
            # t1 = psum/32 + qc ; g1 = tanh(t1) (fp8, into one big tile)
            g1big = scratch.tile([128, KC * RQ], FP8, tag="g1big", bufs=2,
                                 name=f"g1b{sfx}")
            for m in range(KC):
                t1 = scratch.tile([128, RQ], F32, tag="t1", bufs=2,
                                  name=f"t1_{sfx}{m}")
                nc.vector.scalar_tensor_tensor(
                    t1[:, :].rearrange("p (t b) -> p t b", b=BL),
                    zps2[m // 2][:, (m % 2) * RQ:(m % 2 + 1) * RQ].rearrange(
                        "p (t b) -> p t b", b=BL),
                    1.0 / S,
                    qc_sb[:, m * BL:(m + 1) * BL].unsqueeze(1).broadcast_to(
                        [128, TQ, BL]),
                    ALU.mult, ALU.add)
                nc.scalar.activation(
                    g1big[:, m * RQ:(m + 1) * RQ], t1[:, :], AF.Tanh)
                yield
            # W2 pass (fp8 DR) -> G
            gps2 = [pps.tile([128, 512], F32, tag="pzp", name=f"gp{sfx}_{mg}")
                    for mg in range(KC // 2)]
            for mg in range(KC // 2):
                nc.tensor.matmul(gps2[mg][:, :], id_sb[:, :],
                                 zero_sb[:, :], start=True, stop=False,
                                 skip_group_check=True)
            yield
            g1v = g1big[:, :].rearrange("p (k f) -> p k f", k=KC)
            for k2 in range(KC2):
                wt = stream.tile([128, 2 * D], FP8, tag="w2w",
                                 name=f"w2_{sfx}{k2}")
                nc.sync.dma_start(
                    out=wt[:, :], in_=w22[:, k2 * 2 * D:(k2 + 1) * 2 * D])
                yield
                wv = wt[:, :].rearrange("p (two m) -> p two m", two=2)
                for m in range(KC):
                    nc.tensor.matmul(
                        gps2[m // 2][:, (m % 2) * RQ:(m % 2 + 1) * RQ],
                        wv[:, :, m * 128:(m + 1) * 128],
                        g1v[:, 2 * k2:2 * k2 + 2, :],
                        start=False, stop=(k2 == KC2 - 1),
                        perf_mode=DR, skip_group_check=True)
                yield
            gv = g_sb[:, :].rearrange("p (t k b) -> p t k b", k=KC, b=BL)
            for m in range(KC):
                nc.scalar.activation(
                    gv[:, rq * TQ:(rq + 1) * TQ, m, :],
                    gps2[m // 2][:, (m % 2) * RQ:(m % 2 + 1) * RQ].rearrange(
                        "p (t b) -> p t b", b=BL),
                    AF.Sigmoid, scale=1.0 / S, bias=bG_sb[:, m:m + 1])
                yield

        # ---- the scan -------------------------------------------------------
        def scan(nsteps, sfx, bg_gens=()):
            """Attention-GRU scan; returns final h (bf16, [128, KC*BL]).
            bg_gens: list of (step_idx, generator) -- generators advanced
            between scan steps to fill idle engine slots."""
            ps = tc.alloc_tile_pool(name="scanps", bufs=2, space="PSUM")
            h16 = scratch.tile([128, KC * BL], BF16, tag="h16",
                               name=f"h16{sfx}_init", bufs=2)
            nc.vector.memset(h16[:, :], 0.0)
            h8 = None
            giv = gi_n[:, :].rearrange("p (t x) -> p t x", x=KC * BL)
            grzv = gi_rz[:, :].rearrange("p (t x) -> p t x", x=2 * KC * BL)
            gv = g_sb[:, :].rearrange("p (t x) -> p t x", x=KC * BL)
            whhv = whh_sb[:, :].rearrange(
                "p (k two m) -> p k two m", k=KC2, two=2)
            gens = list(bg_gens)

            def advance_bg(n):
                while n > 0 and gens:
                    try:
                        next(gens[0][1])
                        n -= 1
                    except StopIteration:
                        gens.pop(0)

            nb = KC * BL
            seeded = {}

            def seed(t):
                rz = ps.tile([128, 2 * nb], F32, tag="rzps",
                             name=f"rz{sfx}_{t}")
                pn = ps.tile([128, nb], F32, tag="pnps", name=f"pn{sfx}_{t}")
                nc.tensor.matmul(rz[:, :], id_sb[:, :], grzv[:, t, :],
                                 start=True, stop=(t == 0),
                                 skip_group_check=True)
                seeded[t] = (rz, pn)

            seed(0)
            for t in range(nsteps):
                rz, pn = seeded.pop(t)
                if t > 0:
                    hv = h8[:, :].rearrange(
                        "p (k two b) -> p k two b", two=2, b=BL)
                    for m in range(M3):
                        tgt = rz[:, m * BL:(m + 1) * BL] if m < 2 * KC else \
                            pn[:, (m - 2 * KC) * BL:(m - 2 * KC + 1) * BL]
                        for k2 in range(KC2):
                            nc.tensor.matmul(
                                tgt, whhv[:, k2, :, m * 128:(m + 1) * 128],
                                hv[:, k2, :, :],
                                start=(m >= 2 * KC and k2 == 0),
                                stop=(k2 == KC2 - 1),
                                perf_mode=DR, skip_group_check=True)
                else:
                    # h == 0: bank is just the seed; pn is all zero
                    nc.vector.memset(pn[:, :], 0.0)
                if t + 1 < nsteps:
                    seed(t + 1)
                # ACT: one sigmoid -> [rs | 1-z]
                rsz = scratch.tile([128, 2 * nb], BF16, tag="rsz",
                                   name=f"rsz{sfx}_{t}")
                nc.scalar.activation(rsz[:, :], rz[:, :], AF.Sigmoid,
                                     scale=1.0 / S)
                # DVE chain
                n1 = scratch.tile([128, nb], BF16, tag="n1",
                                  name=f"n1{sfx}_{t}")
                nc.vector.scalar_tensor_tensor(
                    n1[:, :], pn[:, :], 1.0 / S, rsz[:, 0:nb],
                    ALU.mult, ALU.mult)
                n2 = scratch.tile([128, nb], BF16, tag="n2",
                                  name=f"n2{sfx}_{t}")
                nc.vector.tensor_tensor(n2[:, :], n1[:, :], giv[:, t, :],
                                        ALU.add)
                gp = scratch.tile([128, nb], BF16, tag="gp",
                                  name=f"gp{sfx}_{t}")
                nc.vector.tensor_tensor(gp[:, :], rsz[:, nb:2 * nb],
                                        gv[:, t, :], ALU.mult)
                hgp = scratch.tile([128, nb], BF16, tag="hgp",
                                   name=f"hgp{sfx}_{t}")
                nc.vector.tensor_tensor(hgp[:, :], h16[:, :], gp[:, :],
                                        ALU.mult)
                hmg = scratch.tile([128, nb], BF16, tag="hmg",
                                   name=f"hmg{sfx}_{t}")
                nc.vector.tensor_tensor(hmg[:, :], h16[:, :], hgp[:, :],
                                        ALU.subtract)
                # ACT: tanh
                nt = scratch.tile([128, nb], BF16, tag="nt",
                                  name=f"nt{sfx}_{t}")
                nc.scalar.activation(nt[:, :], n2[:, :], AF.Tanh)
                # DVE: a1 = nt*gp ; h' = a1 + hmg (fp8 for PE, bf16 carry)
                a1 = scratch.tile([128, nb], BF16, tag="a1",
                                  name=f"a1{sfx}_{t}")
                nc.vector.tensor_tensor(a1[:, :], nt[:, :], gp[:, :],
                                        ALU.mult)
                hn8 = scratch.tile([128, nb], FP8, tag="h8",
                                   name=f"h8{sfx}_{t}", bufs=2)
                nc.vector.tensor_tensor(hn8[:, :], a1[:, :], hmg[:, :],
                                        ALU.add)
                hn16 = scratch.tile([128, nb], BF16, tag="h16",
                                    name=f"h16{sfx}_{t}", bufs=2)
                nc.vector.tensor_tensor(hn16[:, :], a1[:, :], hmg[:, :],
                                        ALU.add)
                h8 = hn8
                h16 = hn16
                advance_bg(_BG_PER_STEP)
            advance_bg(1 << 30)
            ps.release()
            return h16

        def mem_gru(e8_sb, m_sb, m8_sb, sfx):
            """m_new = GRUCell(e, m) with mem weights; feature-major, bf16."""
            ps = tc.alloc_tile_pool(name="memps", bufs=2, space="PSUM")
            nb = KC * BL
            gi_p = ps.tile([128, 3 * nb], F32, tag="memgh", name=f"mgi{sfx}")
            for k in range(KC):
                for g3 in range(3):
                    wt = stream.tile([128, D], BF16, tag="smw",
                                     name=f"mw{sfx}_{k}_{g3}")
                    nc.sync.dma_start(
                        out=wt[:, :],
                        in_=mWihT[k * 128:(k + 1) * 128,
                                  g3 * D:(g3 + 1) * D])
                    for mm in range(KC):
                        m = g3 * KC + mm
                        nc.tensor.matmul(
                            gi_p[:, m * BL:(m + 1) * BL],
                            wt[:, mm * 128:(mm + 1) * 128],
                            e8_sb[:, k * BL:(k + 1) * BL],
                            start=(k == 0 and g3 == 0 and mm == 0),
                            stop=(k == KC - 1 and g3 == 2 and mm == KC - 1))
            gi_f = scratch.tile([128, 3 * nb], F32, tag="memgif",
                                name=f"mgif{sfx}")
            nc.vector.tensor_tensor(
                gi_f[:, :].rearrange("p (m b) -> p m b", b=BL),
                gi_p[:, :].rearrange("p (m b) -> p m b", b=BL),
                bM_sb[:, :].unsqueeze(2).broadcast_to([128, 3 * KC, BL]),
                ALU.add)
            gh_p = ps.tile([128, 3 * nb], F32, tag="memgh", name=f"mgh{sfx}")
            for k in range(KC):
                for g3 in range(3):
                    wt = stream.tile([128, D], BF16, tag="smw",
                                     name=f"mwh{sfx}_{k}_{g3}")
                    nc.sync.dma_start(
                        out=wt[:, :],
                        in_=mWhhT[k * 128:(k + 1) * 128,
                                  g3 * D:(g3 + 1) * D])
                    for mm in range(KC):
                        m = g3 * KC + mm
                        nc.tensor.matmul(
                            gh_p[:, m * BL:(m + 1) * BL],
                            wt[:, mm * 128:(mm + 1) * 128],
                            m8_sb[:, k * BL:(k + 1) * BL],
                            start=(k == 0 and g3 == 0 and mm == 0),
                            stop=(k == KC - 1 and g3 == 2 and mm == KC - 1))
            rz = scratch.tile([128, 2 * nb], F32, tag="mrz", name=f"mrz{sfx}")
            nc.vector.tensor_tensor(
                rz[:, :], gi_f[:, 0:2 * nb], gh_p[:, 0:2 * nb], ALU.add)
            rzs = scratch.tile([128, 2 * nb], F32, tag="mrzs",
                               name=f"mrzs{sfx}")
            nc.scalar.activation(rzs[:, :], rz[:, :], AF.Sigmoid)
            n1 = scratch.tile([128, nb], F32, tag="mn1", name=f"mn1{sfx}")
            nc.vector.tensor_tensor(
                n1[:, :], rzs[:, 0:nb], gh_p[:, 2 * nb:3 * nb], ALU.mult)
            n2 = scratch.tile([128, nb], F32, tag="mn2", name=f"mn2{sfx}")
            nc.vector.tensor_tensor(
                n2[:, :], n1[:, :], gi_f[:, 2 * nb:3 * nb], ALU.add)
            nt = scratch.tile([128, nb], F32, tag="mnt", name=f"mnt{sfx}")
            nc.scalar.activation(nt[:, :], n2[:, :], AF.Tanh)
            d1 = scratch.tile([128, nb], F32, tag="md1", name=f"md1{sfx}")
            nc.vector.tensor_tensor(d1[:, :], m_sb[:, :], nt[:, :],
                                    ALU.subtract)
            d2 = scratch.tile([128, nb], F32, tag="md2", name=f"md2{sfx}")
            nc.vector.tensor_tensor(d2[:, :], d1[:, :], rzs[:, nb:2 * nb],
                                    ALU.mult)
            mn = scratch.tile([128, nb], F32, tag="mnew", bufs=2,
                              name=f"mn{sfx}")
            nc.vector.tensor_tensor(mn[:, :], d2[:, :], nt[:, :], ALU.add)
            mn8 = scratch.tile([128, nb], BF16, tag="mnew8", bufs=2,
                               name=f"mn8{sfx}")
            nc.vector.tensor_copy(mn8[:, :], mn[:, :])
            ps.release()
            return mn, mn8

        # ================= episode 1 (m = q) =================
        # lead-in: P[0] + zg1[0] fully, rest pipelined under scan 1.
        def run_gen(g):
            for _ in g:
                pass

        nrq_used = max(1, min(NRQ, (t_steps * BL + RQ - 1) // RQ))
        run_gen(P_phase(0))
        run_gen(zg_phase(1, 0, qT_sb, wbq_sb, qc1_sb))

        def ep1_bg():
            for rq in range(1, nrq_used):
                for _ in P_phase(rq):
                    yield
                for _ in zg_phase(1, rq, qT_sb, wbq_sb, qc1_sb):
                    yield

        if pipeline:
            h1 = scan(t_steps, "a", bg_gens=[(0, ep1_bg())])
        else:
            run_gen(ep1_bg())
            h1 = scan(t_steps, "a")
        m1, m1_8 = mem_gru(h1, qT_sb, qb_sb, "a")

        # ================= episode 2 (m = m1) =================
        wbm_sb = pool.tile([128, KC * BL], F32, tag="wbm")
        small_matmul(wbT, m1_8, wbm_sb, tagp="swbm")
        qc2a = pool.tile([128, KC * BL], F32, tag="qc2a")
        qc2 = pool.tile([128, KC * BL], F32, tag="qc2")
        small_matmul(w11T, m1_8, qc2a, tagp="sqc2a")
        small_matmul(w12T, qb_sb, qc2, bias=bZ_sb, accum_from=qc2a,
                     tagp="sqc2")

        run_gen(zg_phase(2, 0, m1, wbm_sb, qc2))

        def ep2_bg():
            for rq in range(1, nrq_used):
                for _ in zg_phase(2, rq, m1, wbm_sb, qc2):
                    yield

        if pipeline:
            h2 = scan(t_steps, "b", bg_gens=[(0, ep2_bg())])
        else:
            run_gen(ep2_bg())
            h2 = scan(t_steps, "b")
        m2, _ = mem_gru(h2, m1, m1_8, "b")

        nc.sync.dma_start(out=out[:, :], in_=m2[:, :])

        pps.release()
        for p_ in (scratch, stream, pool):
            p_.release()

    if split_waits:
        _split_multiwait(nc)
    return nc


_BG_PER_STEP = 6


_cache = {}


def _get_nc(t_steps=T):
    if t_steps not in _cache:
        _cache[t_steps] = _build(t_steps)
    return _cache[t_steps]


def _prep_inputs(c, q, Wb_w, W1_w, W1_b, W2_w, W2_b,
                 mem_Wih, mem_Whh, mem_bih, mem_bhh,
                 att_Wih, att_Whh, att_bih, att_bhh):
    """Host-side: transpose/cast/shard everything into per-core in_maps."""
    f32 = np.float32
    c = np.asarray(c, f32); q = np.asarray(q, f32)
    W1j = [np.asarray(W1_w[:, j * D:(j + 1) * D], f32) for j in range(9)]

    def dr_fold(WT, negate_cols=None):
        """[D, Dout] -> [128, KC2*2*Dout] fp8 x32 DoubleRow layout
        [p, (k2, two, dout)]."""
        Ws = np.asarray(WT, f32) * S
        if negate_cols is not None:
            Ws = Ws.copy()
            Ws[:, negate_cols] *= -1.0
        dout = Ws.shape[1]
        return np.ascontiguousarray(
            Ws.reshape(KC2, 2, 128, dout).transpose(2, 0, 1, 3)
            .reshape(128, KC2 * 2 * dout)).astype(fp8_np)

    def dr_fold_multi(blocks):
        """list of [D, D] -> [128, (k2, j, two, D)] fp8 x32."""
        arrs = [np.asarray(b, f32).reshape(KC2, 2, 128, D) * S
                for b in blocks]
        stacked = np.stack(arrs, axis=1)  # [KC2, j, 2, 128, D]
        return np.ascontiguousarray(
            stacked.transpose(3, 0, 1, 2, 4)
            .reshape(128, KC2 * len(blocks) * 2 * D)).astype(fp8_np)

    def fold_bias(v):  # [D] -> [128, KC] (p, m)
        return np.ascontiguousarray(
            np.asarray(v, f32).reshape(KC, 128).T)

    att_b = np.asarray(att_bih, f32) + np.asarray(att_bhh, f32)
    bA32 = np.concatenate([S * att_b[0:D], -S * att_b[D:2 * D]])
    bAn = np.asarray(att_bih, f32)[2 * D:]

    def fold_bias3(bih, bhh):  # [3D] -> [128, 3KC]; bhh only for r,z
        v = np.asarray(bih, f32).copy()
        bhh = np.asarray(bhh, f32)
        v[:2 * D] += bhh[:2 * D]
        return np.ascontiguousarray(v.reshape(3 * KC, 128).T)

    whhT = np.asarray(att_Whh, f32).T  # [D, 3D]
    shared = {
        "w1P": dr_fold_multi([W1j[j].T for j in JP]),
        "w1D": dr_fold_multi([W1j[j].T for j in JD]),
        "wih2": np.ascontiguousarray(
            (np.asarray(att_Wih, f32).T * S)
            .reshape(KC2, 2, 128, 6, 512).transpose(2, 0, 3, 1, 4)
            .reshape(128, KC2 * 6 * 2 * 512)).astype(fp8_np),
        "whh2": dr_fold(whhT, negate_cols=slice(D, 2 * D)),
        "w22": dr_fold(np.asarray(W2_w, f32).T),
        "wbT": np.ascontiguousarray(np.asarray(Wb_w, f32).T).astype(bf16_np),
        "w12sT": np.ascontiguousarray((W1j[1] + W1j[2]).T).astype(bf16_np),
        "w11T": np.ascontiguousarray(W1j[1].T).astype(bf16_np),
        "w12T": np.ascontiguousarray(W1j[2].T).astype(bf16_np),
        "mWihT": np.ascontiguousarray(np.asarray(mem_Wih, f32).T).astype(bf16_np),
        "mWhhT": np.ascontiguousarray(np.asarray(mem_Whh, f32).T).astype(bf16_np),
        "ident": np.eye(128, dtype=f32).astype(bf16_np),
        "bZ": fold_bias(W1_b),
        "bG": fold_bias(W2_b),
        "bA32": np.ascontiguousarray(bA32.reshape(2 * KC, 128).T),
        "bAn": fold_bias(bAn),
        "bM": fold_bias3(mem_bih, mem_bhh),
    }
    assert not np.any(np.asarray(att_bhh, f32)[2 * D:]), \
        "nonzero att_bhh n-gate bias not supported by this kernel build"
    assert not np.any(np.asarray(mem_bhh, f32)[2 * D:]), \
        "nonzero mem_bhh n-gate bias not supported by this kernel build"

    in_maps = []
    for ci in range(NCORES):
        s = ci * BL
        csh = c[:, s:s + BL, :].reshape(ROWS, D)
        qsh = q[s:s + BL, :]
        im = dict(shared)
        im["cT"] = np.ascontiguousarray(csh.T.reshape(KC, 128, ROWS)
                                        .transpose(1, 0, 2)
                                        .reshape(128, KC * ROWS)).astype(bf16_np)
        im["qT"] = np.ascontiguousarray(
            qsh.reshape(BL, KC, 128).transpose(2, 1, 0).reshape(128, KC * BL))
        in_maps.append(im)
    return in_maps


def _unshard(results):
    m = np.empty((B, D), np.float32)
    for ci in range(NCORES):
        o = results[ci]["out"]  # [128, KC*BL]: [p, (k, b)]
        m[ci * BL:(ci + 1) * BL] = (
            o.reshape(128, KC, BL).transpose(2, 1, 0).reshape(BL, D))
    return m


def run_device(in_maps, trace=False):
    nc = _get_nc()
    res = run_bass_kernel_spmd(nc, in_maps, list(range(NCORES)), trace=trace)
    return res


def kernel(**inputs) -> np.ndarray:
    in_maps = _prep_inputs(**inputs)
    res = run_device(in_maps)
    return _unshard(res.results)


if __name__ == "__main__":
    np.random.seed(0)
    pass


# revision 8
# speedup vs baseline: 1.3847x; 1.0050x over previous
"""Trainium2 Bass kernel for nn_EpisodicMemoryModule.

Strategy (v2)
-------------
Math restructure: inside each episode's scan, the gate chain
z -> g1 -> g depends only on (c_t, m, q) -- never on h -- so G[t] is
precomputed with batched matmuls, as is gi_att = c @ att_Wih.T (episode
invariant).  The only sequential work per scan step is
gh = h @ att_Whh.T plus a short elementwise chain.

v2 changes vs v1:
 * All large matmuls run fp8e4m3 with DoubleRow perf mode (256-deep
   contraction per instruction, 0.5 cycles/row).  Weights are scaled by
   32 on the host; descales fold into ACT scale slots.
 * The scan step's per-step bias+gi term is seeded directly into PSUM
   with a single identity matmul, and the z-gate weights/gi are negated
   host-side so ONE sigmoid over the seeded bank yields both r and
   (1-z).  The per-step chain is:
     PE (96 DoubleRow matmuls) -> ACT sigmoid -> DVE n1,n2 -> ACT tanh
     -> DVE a1 -> DVE h'(fp8), with h'(bf16), gp, hgp, hmg off-path.
 * Episode-specific z-delta passes (and the shared P pass) are emitted
   as generators interleaved between scan steps, filling idle PE/DVE/ACT
   slots (the scan is latency-bound, ~35% engine busy).

Sharding: data-parallel over batch B=64 across 8 cores; no inter-core
communication.
"""

import sys

sys.path.insert(0, "/opt/trn_rl_repo")

import numpy as np
import ml_dtypes

import concourse.bass as bass
import concourse.mybir as mybir
from concourse.bass_utils import run_bass_kernel_spmd
from concourse.tile import TileContext
import bass_rust
from bass_rust import ScopedClock

T, B, D = 128, 64, 1024
NCORES = 8
BL = B // NCORES          # 8 batch rows per core
ROWS = T * BL             # 1024 rows per core
KC = D // 128             # 8 contraction chunks of 128
KC2 = D // 256            # 4 contraction chunks of 256 (DoubleRow)
M3 = 3 * D // 128         # 24 output chunks for 3D
RQ = 256                  # row-block for pipelined precompute matmuls
NRQ = ROWS // RQ          # 4
TQ = RQ // BL             # 32 timesteps per row-block
S = 32.0                  # fp8 weight scale

F32 = mybir.dt.float32
BF16 = mybir.dt.bfloat16
FP8 = mybir.dt.float8e4
AF = mybir.ActivationFunctionType
ALU = mybir.AluOpType
DR = mybir.MatmulPerfMode.DoubleRow

bf16_np = ml_dtypes.bfloat16
fp8_np = ml_dtypes.float8_e4m3fn

JP = [0, 3, 5, 7]         # episode-invariant W1 blocks: c, c*q, |c-q|, c*Wbq
JD = [4, 6, 8]            # episode-specific W1 blocks: c*m, |c-m|, c*Wbm


class _TC(TileContext):
    """TileContext whose final drain splits multi-sem waits (the walrus in
    this environment accepts only one sync wait per instruction)."""

    def _drain_and_barrier(self, tick_clock, wait_clock):
        drain_inst = self.nc.sync.drain()
        wait_clock.add_sem_waits(
            drain_inst.ins, ScopedClock({None: tick_clock.global_clock})
        )
        si = drain_inst.ins.sync_info
        if si is not None and si.on_wait and len(si.on_wait) > 1:
            waits = list(si.on_wait)
            drain_inst.ins.sync_info = bass_rust.SyncInfo(
                on_wait=[waits[0]], on_update=list(si.on_update or [])
            )
            for w in waits[1:]:
                d = self.nc.sync.drain()
                d.ins.sync_info = bass_rust.SyncInfo(on_wait=[w], on_update=[])
        self.nc.all_engine_barrier()
        assert self.sems is not None
        popped = self.nc._tile_sem_poison_stack.pop()
        assert popped is self._sem_poison
        self.nc.clear_and_free_semaphores(list(self.sems.allocated().values()))
        self.nc.all_engine_barrier()


def _split_multiwait(nc):
    """Split >1-wait instructions into single-wait NoOps + instruction."""
    nfix = 0
    for f in nc.m.functions:
        for bb in f.blocks:
            insts = list(bb.instructions)
            out = []
            changed = False
            for inst in insts:
                si = inst.sync_info
                if si and si.on_wait and len(si.on_wait) > 1:
                    waits = list(si.on_wait)
                    for i, w in enumerate(waits[:-1]):
                        nop = mybir.InstNoOp(
                            name=f"I-waitfix-{nfix}-{i}", ins=[], outs=[]
                        )
                        nop.engine = inst.engine
                        nop.sync_info = bass_rust.SyncInfo(on_wait=[w], on_update=[])
                        out.append(nop)
                    inst.sync_info = bass_rust.SyncInfo(
                        on_wait=[waits[-1]], on_update=list(si.on_update or [])
                    )
                    nfix += 1
                    changed = True
                out.append(inst)
            if changed:
                bb.instructions = out
    return nfix


def _build(t_steps=T, split_waits=True, pipeline=True):
    """Build the per-core Bass module (SPMD; every core runs the same
    program on its own batch shard)."""
    nc = bass.Bass()
    P = nc.declare_dram_parameter

    # Per-core activations (feature-major) and vectors.
    cT = P("cT", [128, KC * ROWS], BF16, isOutput=False)     # [p,(k rows)]
    qT = P("qT", [128, KC * BL], F32, isOutput=False)        # [p,(k b)]
    # fp8 weights, x32, DoubleRow layouts [p, (.. two ..)].
    w1P = P("w1P", [128, KC2 * len(JP) * 2 * D], FP8, isOutput=False)
    w1D = P("w1D", [128, KC2 * len(JD) * 2 * D], FP8, isOutput=False)
    wih2 = P("wih2", [128, KC2 * 2 * 3 * D], FP8, isOutput=False)
    whh2 = P("whh2", [128, KC2 * 2 * 3 * D], FP8, isOutput=False)  # z negated
    w22 = P("w22", [128, KC2 * 2 * D], FP8, isOutput=False)
    # bf16 weights for the small (BL-row) matmuls.
    wbT = P("wbT", [D, D], BF16, isOutput=False)             # Wb.T
    w12sT = P("w12sT", [D, D], BF16, isOutput=False)         # (W1_1+W1_2).T
    w11T = P("w11T", [D, D], BF16, isOutput=False)           # W1_1.T
    w12T = P("w12T", [D, D], BF16, isOutput=False)           # W1_2.T
    mWihT = P("mWihT", [D, 3 * D], BF16, isOutput=False)
    mWhhT = P("mWhhT", [D, 3 * D], BF16, isOutput=False)
    ident = P("ident", [128, 128], BF16, isOutput=False)
    bZ = P("bZ", [128, KC], F32, isOutput=False)             # W1_b per (p, m)
    bG = P("bG", [128, KC], F32, isOutput=False)             # W2_b
    bA32 = P("bA32", [128, 2 * KC], F32, isOutput=False)     # +-32*(bih+bhh) r,z
    bAn = P("bAn", [128, KC], F32, isOutput=False)           # bih n
    bM = P("bM", [128, 3 * KC], F32, isOutput=False)         # mem bih(+bhh r,z)
    out = P("out", [128, KC * BL], F32, isOutput=True)       # m2.T folded

    with _TC(nc) as tc:
        pool = tc.alloc_tile_pool(name="res", bufs=1)
        stream = tc.alloc_tile_pool(name="stream", bufs=2)
        scratch = tc.alloc_tile_pool(name="scratch", bufs=3)

        # ---- resident loads -------------------------------------------------
        cT_sb = pool.tile([128, KC * ROWS], BF16, tag="cT")    # 16 KB/par
        for k in range(KC):
            nc.sync.dma_start(
                out=cT_sb[:, k * ROWS:(k + 1) * ROWS],
                in_=cT[:, k * ROWS:(k + 1) * ROWS],
            )
        qT_sb = pool.tile([128, KC * BL], F32, tag="qT")
        nc.sync.dma_start(out=qT_sb[:, :], in_=qT[:, :])
        id_sb = pool.tile([128, 128], BF16, tag="id")
        nc.sync.dma_start(out=id_sb[:, :], in_=ident[:, :])
        bZ_sb = pool.tile([128, KC], F32, tag="bZ")
        nc.sync.dma_start(out=bZ_sb[:, :], in_=bZ[:, :])
        bG_sb = pool.tile([128, KC], F32, tag="bG")
        nc.sync.dma_start(out=bG_sb[:, :], in_=bG[:, :])
        bA_sb = pool.tile([128, 2 * KC], F32, tag="bA")
        nc.sync.dma_start(out=bA_sb[:, :], in_=bA32[:, :])
        bAn_sb = pool.tile([128, KC], F32, tag="bAn")
        nc.sync.dma_start(out=bAn_sb[:, :], in_=bAn[:, :])
        bM_sb = pool.tile([128, 3 * KC], F32, tag="bM")
        nc.sync.dma_start(out=bM_sb[:, :], in_=bM[:, :])

        qb_sb = pool.tile([128, KC * BL], BF16, tag="qb")
        nc.vector.tensor_copy(qb_sb[:, :], qT_sb[:, :])

        # fp8 copy of c (matmul rhs for gi / j0 blocks)
        c8_sb = pool.tile([128, KC * ROWS], FP8, tag="c8")     # 8 KB/par
        for k in range(KC):
            nc.vector.tensor_copy(
                c8_sb[:, k * ROWS:(k + 1) * ROWS],
                cT_sb[:, k * ROWS:(k + 1) * ROWS])

        # precomputed per-t data, t-major layouts
        gi_rz = pool.tile([128, T * 2 * KC * BL], BF16, tag="girz")  # 32 KB
        gi_n = pool.tile([128, T * KC * BL], BF16, tag="gin")        # 16 KB
        g_sb = pool.tile([128, T * KC * BL], BF16, tag="G")          # 16 KB
        p_sb = pool.tile([128, KC * ROWS], BF16, tag="P")            # 16 KB (32*P)

        def small_matmul(wT_dram, vec_sb, out_sb, bias=None, accum_from=None,
                         tagp="smallp"):
            """out.T[dout, BL] = W @ vec  (feature-major [128, KC*BL]).
            bf16, one psum accumulation group, chunked weight DMA."""
            ps = tc.alloc_tile_pool(name="smallps", bufs=1, space="PSUM")
            pt = ps.tile([128, KC * BL], F32, tag=tagp, name=f"spt_{tagp}")
            for k in range(KC):
                wt = stream.tile([128, D], BF16, tag="smw", name=f"sw_{tagp}{k}")
                nc.sync.dma_start(
                    out=wt[:, :], in_=wT_dram[k * 128:(k + 1) * 128, :])
                for m in range(KC):
                    nc.tensor.matmul(
                        pt[:, m * BL:(m + 1) * BL],
                        wt[:, m * 128:(m + 1) * 128],
                        vec_sb[:, k * BL:(k + 1) * BL],
                        start=(k == 0 and m == 0),
                        stop=(k == KC - 1 and m == KC - 1),
                    )
            src0 = pt[:, :]
            if bias is not None:
                nc.vector.tensor_tensor(
                    out_sb[:, :].rearrange("p (m b) -> p m b", b=BL),
                    src0.rearrange("p (m b) -> p m b", b=BL),
                    bias[:, :].unsqueeze(2).broadcast_to([128, KC, BL]),
                    ALU.add)
                src0 = out_sb[:, :]
            if accum_from is not None:
                nc.vector.tensor_tensor(
                    out_sb[:, :], src0, accum_from[:, :], ALU.add)
            elif bias is None:
                nc.vector.tensor_copy(out_sb[:, :], src0)
            ps.release()

        # ---- gi phase (lead-in, not pipelined) ------------------------------
        # gi_rz[p, (t, m2, b)] = +-32*(c@Wih.T + bA)[r|z], z negated.
        # gi_n[p, (t, k, b)] = (c@Wih.T + bih)[n].
        def gi_phase():
            ps = tc.alloc_tile_pool(name="gips", bufs=8, space="PSUM")
            c8v = c8_sb[:, :].rearrange("p (k r) -> p k r", k=KC)
            for mg in range(6):            # 4 out-chunks per group
                pts = {}
                for rb in range(2):
                    for mi in range(4):
                        pts[(rb, mi)] = ps.tile(
                            [128, 512], F32, tag="zp", name=f"gip{mg}_{rb}{mi}")
                for k2 in range(KC2):
                    wt = stream.tile([128, 2 * 512], FP8, tag="wih",
                                     name=f"wih{mg}_{k2}")
                    nc.sync.dma_start(
                        out=wt[:, :],
                        in_=wih2[:, (k2 * 6 + mg) * 1024:
                                 (k2 * 6 + mg + 1) * 1024])
                    wv = wt[:, :].rearrange("p (two m) -> p two m", two=2)
                    for rb in range(2):
                        for mi in range(4):
                            nc.tensor.matmul(
                                pts[(rb, mi)][:, :],
                                wv[:, :, mi * 128:(mi + 1) * 128],
                                c8v[:, 2 * k2:2 * k2 + 2,
                                    rb * 512:(rb + 1) * 512],
                                start=(k2 == 0), stop=(k2 == KC2 - 1),
                                perf_mode=DR)
                for rb in range(2):
                    for mi in range(4):
                        m = mg * 4 + mi
                        pv = pts[(rb, mi)][:, :].rearrange(
                            "p (t b) -> p t b", b=BL)
                        if m < 2 * KC:   # r,z: out = +-(psum + 32*bA)
                            sgn = 1.0 if m < KC else -1.0
                            ov = gi_rz[:, :].rearrange(
                                "p (t m b) -> p t m b", m=2 * KC, b=BL)
                            nc.scalar.activation(
                                ov[:, rb * 64:(rb + 1) * 64, m, :], pv,
                                AF.Identity, scale=sgn,
                                bias=bA_sb[:, m:m + 1])
                        else:            # n: out = psum/32 + bih_n
                            mn = m - 2 * KC
                            ov = gi_n[:, :].rearrange(
                                "p (t k b) -> p t k b", k=KC, b=BL)
                            nc.scalar.activation(
                                ov[:, rb * 64:(rb + 1) * 64, mn, :], pv,
                                AF.Identity, scale=1.0 / S,
                                bias=bAn_sb[:, mn:mn + 1])
            ps.release()

        gi_phase()

        # ---- Wbq = q @ Wb.T ; qc1 = q @ (W1_1+W1_2).T + W1_b ---------------
        wbq_sb = pool.tile([128, KC * BL], F32, tag="wbq")
        qc1_sb = pool.tile([128, KC * BL], F32, tag="qc1")
        small_matmul(wbT, qb_sb, wbq_sb, tagp="swbq")
        small_matmul(w12sT, qb_sb, qc1_sb, bias=bZ_sb, tagp="sqc1")

        whh_sb = pool.tile([128, KC2 * 2 * 3 * D], FP8, tag="whh")  # 24 KB/par
        for k2 in range(KC2):
            nc.sync.dma_start(
                out=whh_sb[:, k2 * 6 * D:(k2 + 1) * 6 * D],
                in_=whh2[:, k2 * 6 * D:(k2 + 1) * 6 * D],
            )

        # ---- pipelined P / z-delta passes ----------------------------------
        # Pair accumulators: [128, 512] f32 = one full PSUM bank per tile,
        # holding two adjacent 256-col m-chunks.  Every pair bank is seeded
        # by a single full-bank matmul (start=True) so the 2KB zero-region
        # semantics never poison a half that is mid-accumulation.
        pps = tc.alloc_tile_pool(name="pps", bufs=4, space="PSUM")
        zero_sb = pool.tile([128, 512], BF16, tag="zero")
        nc.vector.memset(zero_sb[:, :], 0.0)
        cT4 = cT_sb[:, :].rearrange("p (k t b) -> p k t b", k=KC, b=BL)
        c8v3 = c8_sb[:, :].rearrange("p (k r) -> p k r", k=KC)

        def make_block2(k2, rq, vecf_sb, wbv_sb, tagsfx):
            """fp8 feature block pairs for 256-chunk k2, row-block rq.
            Each is [128, 2*RQ] laid out (two, t, b)."""
            cview = cT4[:, 2 * k2:2 * k2 + 2, rq * TQ:(rq + 1) * TQ, :]
            vview = vecf_sb[:, :].rearrange("p (k b) -> p k b", b=BL)[
                :, 2 * k2:2 * k2 + 2, :].unsqueeze(2).broadcast_to(
                [128, 2, TQ, BL])
            wview = wbv_sb[:, :].rearrange("p (k b) -> p k b", b=BL)[
                :, 2 * k2:2 * k2 + 2, :].unsqueeze(2).broadcast_to(
                [128, 2, TQ, BL])
            cm = scratch.tile([128, 2 * RQ], FP8, tag="blk_cm", bufs=2,
                              name=f"cm{tagsfx}")
            nc.vector.tensor_tensor(
                cm[:, :].rearrange("p (two t b) -> p two t b", two=2, b=BL),
                cview, vview, ALU.mult)
            tmp = scratch.tile([128, 2 * RQ], BF16, tag="blk_tmp", bufs=2,
                               name=f"bt{tagsfx}")
            nc.vector.tensor_tensor(
                tmp[:, :].rearrange("p (two t b) -> p two t b", two=2, b=BL),
                cview, vview, ALU.subtract)
            am = scratch.tile([128, 2 * RQ], FP8, tag="blk_am", bufs=2,
                              name=f"am{tagsfx}")
            nc.scalar.activation(am[:, :], tmp[:, :], AF.Abs)
            wm = scratch.tile([128, 2 * RQ], FP8, tag="blk_wm", bufs=2,
                              name=f"wm{tagsfx}")
            nc.vector.tensor_tensor(
                wm[:, :].rearrange("p (two t b) -> p two t b", two=2, b=BL),
                cview, wview, ALU.mult)
            return cm, am, wm

        def _pair_rhs(blk):
            return blk[:, :].rearrange("p (two f) -> p two f", two=2)

        def P_phase(rq):
            """P[rq] = 32*(c@W1_0 + (c*q)@W1_3 + |c-q|@W1_5 + (c*Wbq)@W1_7)
            -> p_sb slice (bf16)."""
            zps2 = [pps.tile([128, 512], F32, tag="pzp", name=f"pp{rq}_{mg}")
                    for mg in range(KC // 2)]
            for mg in range(KC // 2):
                nc.tensor.matmul(zps2[mg][:, :], id_sb[:, :],
                                 zero_sb[:, :], start=True, stop=False,
                                 skip_group_check=True)
            yield
            for k2 in range(KC2):
                cm, am, wm = make_block2(k2, rq, qT_sb, wbq_sb, f"P{rq}_{k2}")
                yield
                wt = stream.tile([128, len(JP) * 2 * D], FP8, tag="w1p",
                                 name=f"wP{rq}_{k2}")
                nc.sync.dma_start(
                    out=wt[:, :],
                    in_=w1P[:, k2 * len(JP) * 2 * D:
                            (k2 + 1) * len(JP) * 2 * D])
                yield
                wv = wt[:, :].rearrange("p (j two m) -> p j two m",
                                        j=len(JP), two=2)
                rhss = [c8v3[:, 2 * k2:2 * k2 + 2, rq * RQ:(rq + 1) * RQ],
                        _pair_rhs(cm), _pair_rhs(am), _pair_rhs(wm)]
                for ji in range(len(JP)):
                    for m in range(KC):
                        nc.tensor.matmul(
                            zps2[m // 2][:, (m % 2) * RQ:(m % 2 + 1) * RQ],
                            wv[:, ji, :, m * 128:(m + 1) * 128],
                            rhss[ji],
                            start=False,
                            stop=(k2 == KC2 - 1 and ji == len(JP) - 1),
                            perf_mode=DR, skip_group_check=True)
                    yield
            for m in range(KC):
                nc.scalar.activation(
                    p_sb[:, m * ROWS + rq * RQ: m * ROWS + (rq + 1) * RQ],
                    zps2[m // 2][:, (m % 2) * RQ:(m % 2 + 1) * RQ], AF.Copy)
                yield

        def zg_phase(ep, rq, vecf_sb, wbv_sb, qc_sb):
            """G[rq] for episode ep -> g_sb (t-major slices)."""
            sfx = f"{ep}_{rq}"
            zps2 = [pps.tile([128, 512], F32, tag="pzp", name=f"zd{sfx}_{mg}")
                    for mg in range(KC // 2)]
            p3 = p_sb[:, :].rearrange("p (m r) -> p m r", m=KC)
            for mg in range(KC // 2):
                # seed pair bank with 32*P (one full-bank identity matmul)
                nc.tensor.matmul(
                    zps2[mg][:, :], id_sb[:, :],
                    p3[:, 2 * mg:2 * mg + 2, rq * RQ:(rq + 1) * RQ],
                    start=True, stop=False, skip_group_check=True)
            yield
            for k2 in range(KC2):
                cm, am, wm = make_block2(k2, rq, vecf_sb, wbv_sb,
                                         f"D{sfx}_{k2}")
                yield
                wt = stream.tile([128, len(JD) * 2 * D], FP8, tag="w1d",
                                 name=f"wD{sfx}_{k2}")
                nc.sync.dma_start(
                    out=wt[:, :],
                    in_=w1D[:, k2 * len(JD) * 2 * D:
                            (k2 + 1) * len(JD) * 2 * D])
                yield
                wv = wt[:, :].rearrange("p (j two m) -> p j two m",
                                        j=len(JD), two=2)
                rhss = [_pair_rhs(cm), _pair_rhs(am), _pair_rhs(wm)]
                for ji in range(len(JD)):
                    for m in range(KC):
                        nc.tensor.matmul(
                            zps2[m // 2][:, (m % 2) * RQ:(m % 2 + 1) * RQ],
                            wv[:, ji, :, m * 128:(m + 1) * 128],
                            rhss[ji],
                            start=False,
                            stop=(k2 == KC2 - 1 and ji == len(JD) - 1),
                            perf_mode=DR, skip_group_check=True)
                    yield
            # t1 = psum/32 + qc ; g1 = tanh(t1) (fp8, into one big tile)
            g1big = scratch.tile([128, KC * RQ], FP8, tag="g1big", bufs=2,
                                 name=f"g1b{sfx}")
            for m in range(KC):
                t1 = scratch.tile([128, RQ], F32, tag="t1", bufs=2,
                                  name=f"t1_{sfx}{m}")
                nc.vector.scalar_tensor_tensor(
                    t1[:, :].rearrange("p (t b) -> p t b", b=BL),
                    zps2[m // 2][:, (m % 2) * RQ:(m % 2 + 1) * RQ].rearrange(
                        "p (t b) -> p t b", b=BL),
                    1.0 / S,
                    qc_sb[:, m * BL:(m + 1) * BL].unsqueeze(1).broadcast_to(
                        [128, TQ, BL]),
                    ALU.mult, ALU.add)
                nc.scalar.activation(
                    g1big[:, m * RQ:(m + 1) * RQ], t1[:, :], AF.Tanh)
                yield
            # W2 pass (fp8 DR) -> G
            gps2 = [pps.tile([128, 512], F32, tag="pzp", name=f"gp{sfx}_{mg}")
                    for mg in range(KC // 2)]
            for mg in range(KC // 2):
                nc.tensor.matmul(gps2[mg][:, :], id_sb[:, :],
                                 zero_sb[:, :], start=True, stop=False,
                                 skip_group_check=True)
            yield
            g1v = g1big[:, :].rearrange("p (k f) -> p k f", k=KC)
            for k2 in range(KC2):
                wt = stream.tile([128, 2 * D], FP8, tag="w2w",
                                 name=f"w2_{sfx}{k2}")
                nc.sync.dma_start(
                    out=wt[:, :], in_=w22[:, k2 * 2 * D:(k2 + 1) * 2 * D])
                yield
                wv = wt[:, :].rearrange("p (two m) -> p two m", two=2)
                for m in range(KC):
                    nc.tensor.matmul(
                        gps2[m // 2][:, (m % 2) * RQ:(m % 2 + 1) * RQ],
                        wv[:, :, m * 128:(m + 1) * 128],
                        g1v[:, 2 * k2:2 * k2 + 2, :],
                        start=False, stop=(k2 == KC2 - 1),
                        perf_mode=DR, skip_group_check=True)
                yield
            gv = g_sb[:, :].rearrange("p (t k b) -> p t k b", k=KC, b=BL)
            for m in range(KC):
                nc.scalar.activation(
                    gv[:, rq * TQ:(rq + 1) * TQ, m, :],
                    gps2[m // 2][:, (m % 2) * RQ:(m % 2 + 1) * RQ].rearrange(
                        "p (t b) -> p t b", b=BL),
                    AF.Sigmoid, scale=1.0 / S, bias=bG_sb[:, m:m + 1])
                yield

        # ---- the scan -------------------------------------------------------
        def scan(nsteps, sfx, bg_gens=()):
            """Attention-GRU scan; returns final h (bf16, [128, KC*BL]).
            bg_gens: list of (step_idx, generator) -- generators advanced
            between scan steps to fill idle engine slots."""
            ps = tc.alloc_tile_pool(name="scanps", bufs=2, space="PSUM")
            h16 = scratch.tile([128, KC * BL], BF16, tag="h16",
                               name=f"h16{sfx}_init", bufs=2)
            nc.vector.memset(h16[:, :], 0.0)
            h8 = None
            giv = gi_n[:, :].rearrange("p (t x) -> p t x", x=KC * BL)
            grzv = gi_rz[:, :].rearrange("p (t x) -> p t x", x=2 * KC * BL)
            gv = g_sb[:, :].rearrange("p (t x) -> p t x", x=KC * BL)
            whhv = whh_sb[:, :].rearrange(
                "p (k two m) -> p k two m", k=KC2, two=2)
            gens = list(bg_gens)

            def advance_bg(n):
                while n > 0 and gens:
                    try:
                        next(gens[0][1])
                        n -= 1
                    except StopIteration:
                        gens.pop(0)

            nb = KC * BL
            seeded = {}

            def seed(t):
                rz = ps.tile([128, 2 * nb], F32, tag="rzps",
                             name=f"rz{sfx}_{t}")
                pn = ps.tile([128, nb], F32, tag="pnps", name=f"pn{sfx}_{t}")
                nc.tensor.matmul(rz[:, :], id_sb[:, :], grzv[:, t, :],
                                 start=True, stop=(t == 0),
                                 skip_group_check=True)
                seeded[t] = (rz, pn)

            seed(0)
            for t in range(nsteps):
                rz, pn = seeded.pop(t)
                if t > 0:
                    hv = h8[:, :].rearrange(
                        "p (k two b) -> p k two b", two=2, b=BL)
                    for m in range(M3):
                        tgt = rz[:, m * BL:(m + 1) * BL] if m < 2 * KC else \
                            pn[:, (m - 2 * KC) * BL:(m - 2 * KC + 1) * BL]
                        for k2 in range(KC2):
                            nc.tensor.matmul(
                                tgt, whhv[:, k2, :, m * 128:(m + 1) * 128],
                                hv[:, k2, :, :],
                                start=(m >= 2 * KC and k2 == 0),
                                stop=(k2 == KC2 - 1),
                                perf_mode=DR, skip_group_check=True)
                else:
                    # h == 0: bank is just the seed; pn is all zero
                    nc.vector.memset(pn[:, :], 0.0)
                if t + 1 < nsteps:
                    seed(t + 1)
                # ACT: one sigmoid -> [rs | 1-z]
                rsz = scratch.tile([128, 2 * nb], BF16, tag="rsz",
                                   name=f"rsz{sfx}_{t}")
                nc.scalar.activation(rsz[:, :], rz[:, :], AF.Sigmoid,
                                     scale=1.0 / S)
                # DVE chain
                n1 = scratch.tile([128, nb], BF16, tag="n1",
                                  name=f"n1{sfx}_{t}")
                nc.vector.scalar_tensor_tensor(
                    n1[:, :], pn[:, :], 1.0 / S, rsz[:, 0:nb],
                    ALU.mult, ALU.mult)
                n2 = scratch.tile([128, nb], BF16, tag="n2",
                                  name=f"n2{sfx}_{t}")
                nc.vector.tensor_tensor(n2[:, :], n1[:, :], giv[:, t, :],
                                        ALU.add)
                gp = scratch.tile([128, nb], BF16, tag="gp",
                                  name=f"gp{sfx}_{t}")
                nc.vector.tensor_tensor(gp[:, :], rsz[:, nb:2 * nb],
                                        gv[:, t, :], ALU.mult)
                hgp = scratch.tile([128, nb], BF16, tag="hgp",
                                   name=f"hgp{sfx}_{t}")
                nc.vector.tensor_tensor(hgp[:, :], h16[:, :], gp[:, :],
                                        ALU.mult)
                hmg = scratch.tile([128, nb], BF16, tag="hmg",
                                   name=f"hmg{sfx}_{t}")
                nc.vector.tensor_tensor(hmg[:, :], h16[:, :], hgp[:, :],
                                        ALU.subtract)
                # ACT: tanh
                nt = scratch.tile([128, nb], BF16, tag="nt",
                                  name=f"nt{sfx}_{t}")
                nc.scalar.activation(nt[:, :], n2[:, :], AF.Tanh)
                # DVE: a1 = nt*gp ; h' = a1 + hmg (fp8 for PE, bf16 carry)
                a1 = scratch.tile([128, nb], BF16, tag="a1",
                                  name=f"a1{sfx}_{t}")
                nc.vector.tensor_tensor(a1[:, :], nt[:, :], gp[:, :],
                                        ALU.mult)
                hn8 = scratch.tile([128, nb], FP8, tag="h8",
                                   name=f"h8{sfx}_{t}", bufs=2)
                nc.vector.tensor_tensor(hn8[:, :], a1[:, :], hmg[:, :],
                                        ALU.add)
                hn16 = scratch.tile([128, nb], BF16, tag="h16",
                                    name=f"h16{sfx}_{t}", bufs=2)
                nc.vector.tensor_tensor(hn16[:, :], a1[:, :], hmg[:, :],
                                        ALU.add)
                h8 = hn8
                h16 = hn16
                advance_bg(_BG_PER_STEP)
            advance_bg(1 << 30)
            ps.release()
            return h16

        def mem_gru(e8_sb, m_sb, m8_sb, sfx):
            """m_new = GRUCell(e, m) with mem weights; feature-major, bf16."""
            ps = tc.alloc_tile_pool(name="memps", bufs=2, space="PSUM")
            nb = KC * BL
            gi_p = ps.tile([128, 3 * nb], F32, tag="memgh", name=f"mgi{sfx}")
            for k in range(KC):
                for g3 in range(3):
                    wt = stream.tile([128, D], BF16, tag="smw",
                                     name=f"mw{sfx}_{k}_{g3}")
                    nc.sync.dma_start(
                        out=wt[:, :],
                        in_=mWihT[k * 128:(k + 1) * 128,
                                  g3 * D:(g3 + 1) * D])
                    for mm in range(KC):
                        m = g3 * KC + mm
                        nc.tensor.matmul(
                            gi_p[:, m * BL:(m + 1) * BL],
                            wt[:, mm * 128:(mm + 1) * 128],
                            e8_sb[:, k * BL:(k + 1) * BL],
                            start=(k == 0 and g3 == 0 and mm == 0),
                            stop=(k == KC - 1 and g3 == 2 and mm == KC - 1))
            gi_f = scratch.tile([128, 3 * nb], F32, tag="memgif",
                                name=f"mgif{sfx}")
            nc.vector.tensor_tensor(
                gi_f[:, :].rearrange("p (m b) -> p m b", b=BL),
                gi_p[:, :].rearrange("p (m b) -> p m b", b=BL),
                bM_sb[:, :].unsqueeze(2).broadcast_to([128, 3 * KC, BL]),
                ALU.add)
            gh_p = ps.tile([128, 3 * nb], F32, tag="memgh", name=f"mgh{sfx}")
            for k in range(KC):
                for g3 in range(3):
                    wt = stream.tile([128, D], BF16, tag="smw",
                                     name=f"mwh{sfx}_{k}_{g3}")
                    nc.sync.dma_start(
                        out=wt[:, :],
                        in_=mWhhT[k * 128:(k + 1) * 128,
                                  g3 * D:(g3 + 1) * D])
                    for mm in range(KC):
                        m = g3 * KC + mm
                        nc.tensor.matmul(
                            gh_p[:, m * BL:(m + 1) * BL],
                            wt[:, mm * 128:(mm + 1) * 128],
                            m8_sb[:, k * BL:(k + 1) * BL],
                            start=(k == 0 and g3 == 0 and mm == 0),
                            stop=(k == KC - 1 and g3 == 2 and mm == KC - 1))
            rz = scratch.tile([128, 2 * nb], F32, tag="mrz", name=f"mrz{sfx}")
            nc.vector.tensor_tensor(
                rz[:, :], gi_f[:, 0:2 * nb], gh_p[:, 0:2 * nb], ALU.add)
            rzs = scratch.tile([128, 2 * nb], F32, tag="mrzs",
                               name=f"mrzs{sfx}")
            nc.scalar.activation(rzs[:, :], rz[:, :], AF.Sigmoid)
            n1 = scratch.tile([128, nb], F32, tag="mn1", name=f"mn1{sfx}")
            nc.vector.tensor_tensor(
                n1[:, :], rzs[:, 0:nb], gh_p[:, 2 * nb:3 * nb], ALU.mult)
            n2 = scratch.tile([128, nb], F32, tag="mn2", name=f"mn2{sfx}")
            nc.vector.tensor_tensor(
                n2[:, :], n1[:, :], gi_f[:, 2 * nb:3 * nb], ALU.add)
            nt = scratch.tile([128, nb], F32, tag="mnt", name=f"mnt{sfx}")
            nc.scalar.activation(nt[:, :], n2[:, :], AF.Tanh)
            d1 = scratch.tile([128, nb], F32, tag="md1", name=f"md1{sfx}")
            nc.vector.tensor_tensor(d1[:, :], m_sb[:, :], nt[:, :],
                                    ALU.subtract)
            d2 = scratch.tile([128, nb], F32, tag="md2", name=f"md2{sfx}")
            nc.vector.tensor_tensor(d2[:, :], d1[:, :], rzs[:, nb:2 * nb],
                                    ALU.mult)
            mn = scratch.tile([128, nb], F32, tag="mnew", bufs=2,
                              name=f"mn{sfx}")
            nc.vector.tensor_tensor(mn[:, :], d2[:, :], nt[:, :], ALU.add)
            mn8 = scratch.tile([128, nb], BF16, tag="mnew8", bufs=2,
                               name=f"mn8{sfx}")
            nc.vector.tensor_copy(mn8[:, :], mn[:, :])
            ps.release()
            return mn, mn8

        # ================= episode 1 (m = q) =================
        # lead-in: P[0] + zg1[0] fully, rest pipelined under scan 1.
        def run_gen(g):
            for _ in g:
                pass

        nrq_used = max(1, min(NRQ, (t_steps * BL + RQ - 1) // RQ))
        run_gen(P_phase(0))
        run_gen(zg_phase(1, 0, qT_sb, wbq_sb, qc1_sb))

        def ep1_bg():
            for rq in range(1, nrq_used):
                for _ in P_phase(rq):
                    yield
                for _ in zg_phase(1, rq, qT_sb, wbq_sb, qc1_sb):
                    yield

        if pipeline:
            h1 = scan(t_steps, "a", bg_gens=[(0, ep1_bg())])
        else:
            run_gen(ep1_bg())
            h1 = scan(t_steps, "a")
        m1, m1_8 = mem_gru(h1, qT_sb, qb_sb, "a")

        # ================= episode 2 (m = m1) =================
        wbm_sb = pool.tile([128, KC * BL], F32, tag="wbm")
        small_matmul(wbT, m1_8, wbm_sb, tagp="swbm")
        qc2a = pool.tile([128, KC * BL], F32, tag="qc2a")
        qc2 = pool.tile([128, KC * BL], F32, tag="qc2")
        small_matmul(w11T, m1_8, qc2a, tagp="sqc2a")
        small_matmul(w12T, qb_sb, qc2, bias=bZ_sb, accum_from=qc2a,
                     tagp="sqc2")

        run_gen(zg_phase(2, 0, m1, wbm_sb, qc2))

        def ep2_bg():
            for rq in range(1, nrq_used):
                for _ in zg_phase(2, rq, m1, wbm_sb, qc2):
                    yield

        if pipeline:
            h2 = scan(t_steps, "b", bg_gens=[(0, ep2_bg())])
        else:
            run_gen(ep2_bg())
            h2 = scan(t_steps, "b")
        m2, _ = mem_gru(h2, m1, m1_8, "b")

        nc.sync.dma_start(out=out[:, :], in_=m2[:, :])

        pps.release()
        for p_ in (scratch, stream, pool):
            p_.release()

    if split_waits:
        _split_multiwait(nc)
    return nc


_BG_PER_STEP = 6


_cache = {}


def _get_nc(t_steps=T):
    if t_steps not in _cache:
        _cache[t_steps] = _build(t_steps)
    return _cache[t_steps]


def _prep_inputs(c, q, Wb_w, W1_w, W1_b, W2_w, W2_b,
                 mem_Wih, mem_Whh, mem_bih, mem_bhh,
                 att_Wih, att_Whh, att_bih, att_bhh):
    """Host-side: transpose/cast/shard everything into per-core in_maps."""
    f32 = np.float32
    c = np.asarray(c, f32); q = np.asarray(q, f32)
    W1j = [np.asarray(W1_w[:, j * D:(j + 1) * D], f32) for j in range(9)]

    def dr_fold(WT, negate_cols=None):
        """[D, Dout] -> [128, KC2*2*Dout] fp8 x32 DoubleRow layout
        [p, (k2, two, dout)]."""
        Ws = np.asarray(WT, f32) * S
        if negate_cols is not None:
            Ws = Ws.copy()
            Ws[:, negate_cols] *= -1.0
        dout = Ws.shape[1]
        return np.ascontiguousarray(
            Ws.reshape(KC2, 2, 128, dout).transpose(2, 0, 1, 3)
            .reshape(128, KC2 * 2 * dout)).astype(fp8_np)

    def dr_fold_multi(blocks):
        """list of [D, D] -> [128, (k2, j, two, D)] fp8 x32."""
        arrs = [np.asarray(b, f32).reshape(KC2, 2, 128, D) * S
                for b in blocks]
        stacked = np.stack(arrs, axis=1)  # [KC2, j, 2, 128, D]
        return np.ascontiguousarray(
            stacked.transpose(3, 0, 1, 2, 4)
            .reshape(128, KC2 * len(blocks) * 2 * D)).astype(fp8_np)

    def fold_bias(v):  # [D] -> [128, KC] (p, m)
        return np.ascontiguousarray(
            np.asarray(v, f32).reshape(KC, 128).T)

    att_b = np.asarray(att_bih, f32) + np.asarray(att_bhh, f32)
    bA32 = np.concatenate([S * att_b[0:D], -S * att_b[D:2 * D]])
    bAn = np.asarray(att_bih, f32)[2 * D:]

    def fold_bias3(bih, bhh):  # [3D] -> [128, 3KC]; bhh only for r,z
        v = np.asarray(bih, f32).copy()
        bhh = np.asarray(bhh, f32)
        v[:2 * D] += bhh[:2 * D]
        return np.ascontiguousarray(v.reshape(3 * KC, 128).T)

    whhT = np.asarray(att_Whh, f32).T  # [D, 3D]
    shared = {
        "w1P": dr_fold_multi([W1j[j].T for j in JP]),
        "w1D": dr_fold_multi([W1j[j].T for j in JD]),
        "wih2": np.ascontiguousarray(
            (np.asarray(att_Wih, f32).T * S)
            .reshape(KC2, 2, 128, 6, 512).transpose(2, 0, 3, 1, 4)
            .reshape(128, KC2 * 6 * 2 * 512)).astype(fp8_np),
        "whh2": dr_fold(whhT, negate_cols=slice(D, 2 * D)),
        "w22": dr_fold(np.asarray(W2_w, f32).T),
        "wbT": np.ascontiguousarray(np.asarray(Wb_w, f32).T).astype(bf16_np),
        "w12sT": np.ascontiguousarray((W1j[1] + W1j[2]).T).astype(bf16_np),
        "w11T": np.ascontiguousarray(W1j[1].T).astype(bf16_np),
        "w12T": np.ascontiguousarray(W1j[2].T).astype(bf16_np),
        "mWihT": np.ascontiguousarray(np.asarray(mem_Wih, f32).T).astype(bf16_np),
        "mWhhT": np.ascontiguousarray(np.asarray(mem_Whh, f32).T).astype(bf16_np),
        "ident": np.eye(128, dtype=f32).astype(bf16_np),
        "bZ": fold_bias(W1_b),
        "bG": fold_bias(W2_b),
        "bA32": np.ascontiguousarray(bA32.reshape(2 * KC, 128).T),
        "bAn": fold_bias(bAn),
        "bM": fold_bias3(mem_bih, mem_bhh),
    }
    assert not np.any(np.asarray(att_bhh, f32)[2 * D:]), \
        "nonzero att_bhh n-gate bias not supported by this kernel build"
    assert not np.any(np.asarray(mem_bhh, f32)[2 * D:]), \
        "nonzero mem_bhh n-gate bias not supported by this kernel build"

    in_maps = []
    for ci in range(NCORES):
        s = ci * BL
        csh = c[:, s:s + BL, :].reshape(ROWS, D)
        qsh = q[s:s + BL, :]
        im = dict(shared)
        im["cT"] = np.ascontiguousarray(csh.T.reshape(KC, 128, ROWS)
                                        .transpose(1, 0, 2)
                                        .reshape(128, KC * ROWS)).astype(bf16_np)
        im["qT"] = np.ascontiguousarray(
            qsh.reshape(BL, KC, 128).transpose(2, 1, 0).reshape(128, KC * BL))
        in_maps.append(im)
    return in_maps


def _unshard(results):
    m = np.empty((B, D), np.float32)
    for ci in range(NCORES):
        o = results[ci]["out"]  # [128, KC*BL]: [p, (k, b)]
        m[ci * BL:(ci + 1) * BL] = (
            o.reshape(128, KC, BL).transpose(2, 1, 0).reshape(BL, D))
    return m


def run_device(in_maps, trace=False):
    nc = _get_nc()
    res = run_bass_kernel_spmd(nc, in_maps, list(range(NCORES)), trace=trace)
    return res


def kernel(**inputs) -> np.ndarray:
    in_maps = _prep_inputs(**inputs)
    res = run_device(in_maps)
    return _unshard(res.results)


if __name__ == "__main__":
    np.random.seed(0)
    pass
